# revision 1
# baseline (speedup 1.0000x reference)
"""Trainium2 Bass kernel for nn_EstimatorQNN.

Math reduction: the reference applies a batch-independent 2x2 unitary U
(built from the 4 weights) to |psi> = [cos(th/2), sin(th/2)] with
th = x0 + x1, then returns |amp0|^2 - |amp1|^2.  By unitarity this
collapses to

    out = A*cos(th) + D*sin(th) = R*sin(th + phi)

with A = 2|U00|^2 - 1, D = 2*Re(U00*conj(U01)), R = hypot(A, D),
phi = atan2(A, D).  A/D/R/phi are 4 scalars computed on host from the
weights; the device does the memory-bound elementwise part.

Device chain per element (HW Sin table is only valid on [-pi, pi], so
range-reduce with the fp32 magic-number round trick):
    th' = (x_even + phi) + x_odd              scalar_tensor_tensor   (DVE)
    m   = th'*(1/2pi) + MAGIC                 tensor_scalar (DVE) or
                                              activation Identity (ACT)
    k2  = (m - MAGIC)*2pi                     tensor_scalar          (DVE)
    psi = th' - k2                            tensor_tensor          (DVE)
    s   = Sin(psi)                            activation             (ACT)
    y   = s * R                               activation Copy        (ACT)

Raw-Bass hand-scheduled pipeline (no Tile framework).  Loads are many
small DMAs strictly alternating between the two HWDGE rings (per-ring
FIFO then delivers tiles at the aggregate HBM rate, so the DVE never
starves); compute runs on fewer, larger column-blocks of one SBUF input
arena (fewer per-op fixed costs); the m-op of late blocks runs on ACT to
balance DVE; stores go out on the sync ring and the idle GpSimd SWDGE
ring so the scalar sequencer only carries its ring's loads.  A global op
plan is linearized and every RAW/WAR/WAW hazard gets an explicit
semaphore wait (TRN2 engine pipelines are deep; even same-engine readers
must sem-wait on the writer).  Pure data parallel over 8 NeuronCores.
"""

import math
from contextlib import ExitStack

import numpy as np

B_FULL = 8388608
N_CORES = 8
B_SHARD = B_FULL // N_CORES  # 1048576

LOAD_COLS = [1024, 1024, 1024, 1024, 2048, 2048, 2048, 2048, 2048, 1024, 1024]
assert sum(LOAD_COLS) * 128 == B_SHARD * 2
BLOCKS = [(0,), (1,), (2, 3), (4,), (5,), (6, 7), (8, 9), (10,)]  # load idxs/blk
# stores: early/mid on the sync HWDGE ring (its loads finish by then) and
# the gpsimd SWDGE ring; late stores on the ACT ring, which is empty once
# its loads are done
STORE_RING = ["s", "g", "s", "s", "s", "a", "a", "a"]
MUL_ON_DVE = {7}                   # last block's R-multiply runs on idle DVE
# NOTE: offloading the m-op to ACT was tried three ways (early blocks, late
# blocks, software-pipelined) and always measured slower: ACT pays ~0.7us of
# pipeline-drain per same-engine dependent op, so its effective throughput is
# far below its busy-sum.  ACT carries only sin + mul.
M_ON_ACT = set()

MAGIC = 12582912.0                 # 1.5 * 2**23: fp32 round-to-nearest-int
TWO_PI = 6.283185307179586
INV_2PI = 1.0 / TWO_PI

LAST_RESULT = None


def _host_constants(weights: np.ndarray):
    w = np.asarray(weights, dtype=np.float64)

    def rx(t):
        c, s = np.cos(t / 2), np.sin(t / 2)
        return np.array([[c, -1j * s], [-1j * s, c]], dtype=np.complex128)

    def rz(t):
        return np.array(
            [[np.exp(-1j * t / 2), 0], [0, np.exp(1j * t / 2)]], dtype=np.complex128
        )

    U = np.eye(2, dtype=np.complex128)
    for i in range(len(w) // 2):
        U = rz(w[2 * i + 1]) @ rx(w[2 * i]) @ U
    A = 2.0 * abs(U[0, 0]) ** 2 - 1.0
    D = 2.0 * (U[0, 0] * np.conj(U[0, 1])).real
    R = math.hypot(A, D)
    phi = math.atan2(A, D)
    return float(R), float(phi)


def _plan_waits(plan):
    """Assign per-op semaphore waits for every RAW/WAR/WAW hazard."""
    semval = {}
    writer = {}
    readers = {}
    seen = {}
    for op in plan:
        want = {}
        for b in op["reads"]:
            if b in writer:
                s, v = writer[b]
                want[s] = max(want.get(s, 0), v)
        for b in op["writes"]:
            for s, v in readers.get(b, []):
                want[s] = max(want.get(s, 0), v)
            if b in writer:
                s, v = writer[b]
                want[s] = max(want.get(s, 0), v)
        eng_seen = seen.setdefault(op["eng"], {})
        waits = []
        for s, v in want.items():
            if eng_seen.get(s, -1) < v:
                waits.append((s, v))
                eng_seen[s] = v
        op["waits"] = waits
        semval[op["sem"]] = semval.get(op["sem"], 0) + op["inc"]
        point = (op["sem"], semval[op["sem"]])
        for b in op["writes"]:
            writer[b] = point
            readers[b] = []
        for b in op["reads"]:
            readers.setdefault(b, []).append(point)
    return plan


def _build_nc(R: float, phi: float):
    import concourse.bacc as bacc
    from concourse import mybir

    add = mybir.AluOpType.add
    sub = mybir.AluOpType.subtract
    mult = mybir.AluOpType.mult
    f32 = mybir.dt.float32
    Sin = mybir.ActivationFunctionType.Sin
    Identity = mybir.ActivationFunctionType.Identity

    nc = bacc.Bacc(
        "TRN2",
        target_bir_lowering=False,
        debug=False,
        enable_asserts=False,
        num_devices=N_CORES,
    )
    x = nc.dram_tensor("x", [B_SHARD, 2], f32, kind="ExternalInput").ap()
    y = nc.dram_tensor("y", [B_SHARD, 1], f32, kind="ExternalOutput").ap()
    xf = x.rearrange("n t -> (n t)")
    yf = y.rearrange("n o -> (n o)")

    n_loads = len(LOAD_COLS)
    n_blocks = len(BLOCKS)
    TOT_COLS = sum(LOAD_COLS)                 # 16384
    lcol = [sum(LOAD_COLS[:i]) for i in range(n_loads)]       # col offsets
    bcols = [sum(LOAD_COLS[a] for a in blk) for blk in BLOCKS]
    boff = [lcol[blk[0]] for blk in BLOCKS]

    # DRAM views.  The SBUF input arena is [128, TOT_COLS]; partition p of
    # the arena holds input flat [p*TOT_COLS, (p+1)*TOT_COLS).  Load j
    # fills arena cols [lcol[j], lcol[j]+LOAD_COLS[j]) from the matching
    # DRAM stripe (per-partition contiguous runs of LOAD_COLS[j] floats).
    xin = [
        xf.rearrange("(p c) -> p c", p=128)[:, lcol[j] : lcol[j] + LOAD_COLS[j]]
        for j in range(n_loads)
    ]
    yout = [
        yf.rearrange("(p c) -> p c", p=128)[:, boff[b] // 2 : (boff[b] + bcols[b]) // 2]
        for b in range(n_blocks)
    ]

    HMAX = max(bcols) // 2

    arena = nc.alloc_sbuf_tensor("arena", [128, TOT_COLS], f32)
    o_bufs = [nc.alloc_sbuf_tensor(f"o{b}", [128, bcols[b] // 2], f32) for b in range(n_blocks)]
    th = [nc.alloc_sbuf_tensor(f"th{j}", [128, HMAX], f32) for j in range(2)]
    mt = [nc.alloc_sbuf_tensor(f"mt{j}", [128, HMAX], f32) for j in range(2)]
    k2 = [nc.alloc_sbuf_tensor(f"k2{j}", [128, HMAX], f32) for j in range(2)]
    psi = [nc.alloc_sbuf_tensor(f"psi{j}", [128, HMAX], f32) for j in range(2)]
    sb = [nc.alloc_sbuf_tensor(f"s{j}", [128, HMAX], f32) for j in range(2)]
    magic = nc.alloc_sbuf_tensor("magic", [128, 1], f32)

    # ---- phase 1: global plan --------------------------------------------
    def op(eng, kind, i, reads, writes, sem, inc=1):
        return dict(eng=eng, kind=kind, i=i, reads=reads, writes=writes,
                    sem=sem, inc=inc)

    plan = []
    for j in range(n_loads):
        ring = "s" if j % 2 == 0 else "a"
        plan.append(op(ring, "load", j, [], [f"t{j}"], f"l{j}", 16))
    plan.append(op("v", "memset", 0, [], ["magic"], "vq"))

    def blk_reads(b):
        return [f"t{a}" for a in BLOCKS[b]]

    def dve_front(b, with_m):
        plan.append(op("v", "stt", b, blk_reads(b), [f"th{b % 2}"], "vq"))

    def dve_tail(b):
        # range-reduce th+phi into [-pi, pi] with two cascaded single-op
        # conditional 2pi-wraps (custom DVE op); one wrap only covers
        # |th'| <= 3pi and ~1e-6 of a randn batch exceeds that
        plan.append(op("v", "w1", b, [f"th{b % 2}"], [f"mt{b % 2}"], "vq"))
        plan.append(op("v", "w2", b, [f"mt{b % 2}"], [f"psi{b % 2}"], "vq"))

    def act_blk(b):
        plan.append(op("a", "sin", b, [f"psi{b % 2}"], [f"s{b % 2}"], "aq"))
        if b in MUL_ON_DVE:
            plan.append(op("v", "mul", b, [f"s{b % 2}"], [f"o{b}"], "vq"))
        else:
            plan.append(op("a", "mul", b, [f"s{b % 2}"], [f"o{b}"], "aq"))
        plan.append(op(STORE_RING[b], "store", b, [f"o{b}"], [], f"os{b}", 16))

    for b in range(len(BLOCKS)):
        dve_front(b, with_m=True)
        dve_tail(b)
        act_blk(b)

    _plan_waits(plan)

    # ---- phase 2: emit per-engine streams --------------------------------
    with ExitStack() as ctx:
        sems = {}
        for o in plan:
            if o["sem"] not in sems:
                sems[o["sem"]] = ctx.enter_context(nc.semaphore(o["sem"]))
        block = ctx.enter_context(nc.Block())

        def emit(o, eng):
            for s, v in o["waits"]:
                eng.wait_ge(sems[s], v)
            i = o["i"]
            k = o["kind"]
            if k == "load":
                inst = eng.dma_start(
                    arena.ap()[:, lcol[i] : lcol[i] + LOAD_COLS[i]], xin[i]
                )
            elif k == "store":
                inst = eng.dma_start(yout[i], o_bufs[i].ap())
            elif k == "memset":
                inst = nc.vector.memset(magic.ap(), MAGIC)
            else:
                h = bcols[i] // 2
                j = i % 2
                if k == "stt":
                    t = arena.ap()[:, boff[i] : boff[i] + bcols[i]]
                    inst = nc.vector.scalar_tensor_tensor(
                        th[j].ap()[:, :h], t[:, 0 : 2 * h : 2], phi,
                        t[:, 1 : 2 * h : 2], op0=add, op1=add,
                    )
                elif k == "w1":
                    inst = nc.vector.add_range_wrap(
                        mt[j].ap()[:, :h], th[j].ap()[:, :h],
                        0.0, 3.1415927410125732, TWO_PI,
                    )
                elif k == "w2":
                    inst = nc.vector.add_range_wrap(
                        psi[j].ap()[:, :h], mt[j].ap()[:, :h],
                        0.0, 3.1415927410125732, TWO_PI,
                    )
                elif k == "sin":
                    inst = nc.scalar.activation(
                        sb[j].ap()[:, :h], psi[j].ap()[:, :h], Sin,
                        bias=0.0, scale=1.0,
                    )
                elif k == "mul" and o["eng"] == "v":
                    inst = nc.vector.tensor_scalar_mul(
                        o_bufs[i].ap(), sb[j].ap()[:, :h], R
                    )
                elif k == "mul":
                    inst = nc.scalar.mul(o_bufs[i].ap(), sb[j].ap()[:, :h], R)
                else:
                    raise AssertionError(k)
            inst.then_inc(sems[o["sem"]], o["inc"])

        @block.sync
        def _(sync):
            for o in plan:
                if o["eng"] == "s":
                    emit(o, sync)
            for b in range(n_blocks):
                if STORE_RING[b] == "s":
                    sync.wait_ge(sems[f"os{b}"], 16)

        @block.vector
        def _(vector):
            for o in plan:
                if o["eng"] == "v":
                    emit(o, vector)

        @block.gpsimd
        def _(gpsimd):
            for o in plan:
                if o["eng"] == "g":
                    emit(o, gpsimd)
            for b in range(n_blocks):
                if STORE_RING[b] == "g":
                    gpsimd.wait_ge(sems[f"os{b}"], 16)

        @block.scalar
        def _(scalar):
            for o in plan:
                if o["eng"] == "a":
                    emit(o, scalar)
            for b in range(n_blocks):
                if STORE_RING[b] == "a":
                    scalar.wait_ge(sems[f"os{b}"], 16)

    nc.compile()
    return nc


def kernel(inputs: np.ndarray, weights: np.ndarray, _trace: bool = False) -> np.ndarray:
    global LAST_RESULT
    from concourse.bass_utils import run_bass_kernel_spmd

    inputs = np.ascontiguousarray(np.asarray(inputs, dtype=np.float32))
    assert inputs.shape == (B_FULL, 2), inputs.shape

    R, phi = _host_constants(weights)
    nc = _build_nc(R, phi)

    in_maps = [
        {"x": inputs[c * B_SHARD : (c + 1) * B_SHARD]} for c in range(N_CORES)
    ]
    res = run_bass_kernel_spmd(
        nc, in_maps, core_ids=list(range(N_CORES)), trace=_trace
    )
    LAST_RESULT = res
    out = np.concatenate([r["y"] for r in res.results], axis=0)
    return out.astype(np.float32, copy=False)



# revision 4
# speedup vs baseline: 1.3966x; 1.3966x over previous
"""Trainium2 Bass kernel for nn_EstimatorQNN.

Math reduction: the reference applies a batch-independent 2x2 unitary U
(built from the 4 weights) to |psi> = [cos(th/2), sin(th/2)] with
th = x0 + x1, then returns |amp0|^2 - |amp1|^2.  By unitarity this
collapses to

    out = R*sin(th + phi)

with R = hypot(A, D), phi = atan2(A, D), A = 2|U00|^2 - 1,
D = 2*Re(U00*conj(U01)) -- 2 scalars computed on host from the weights;
the device does the memory-bound elementwise part.

This version halves HBM traffic vs the f32 baseline by staging the
inputs as de-interleaved fp16 streams (x0, x1 contiguous) and storing
fp16 (host widens to f32; tolerance budget is ~2e-2 and fp16 costs
~5e-4).  Per core: 4 MB loads + 2 MB stores = 6 MB -> ~17 us HBM floor
at 358 GB/s.

Compute is two fused custom-DVE ops (registered via the documented
dve_ops.OPS extension point; shas self-pinned at import):

  QNN_SUMWRAP:  f = t2 - rne_round(t2),  t2 = (x0+x1)*(1/2pi) + phi/2pi
                (round via the fp32 +/-1.5*2^23 magic trick).  f is the
                angle in *turns*, exactly in [-0.5, 0.5] for all inputs
                -- no range-reduction tail cases at all.
  QNN_SINPOLY:  y = R*sin(2pi*f) as an odd minimax degree-7 polynomial
                (R folded into the coefficients, max err 2.5e-4).

Per 2048-col block: DVE runs SUMWRAP; then either ACT evaluates
sin (table, with the free input scale=2pi) + R-mul, or DVE evaluates
SINPOLY (blocks in DVE_SIN) so neither engine exceeds the DMA floor.
Raw-Bass hand-scheduled: loads split across the two HWDGE rings,
stores on sync/gpsimd/act rings, explicit semaphore waits for every
hazard (deep TRN2 pipelines need same-engine waits too).
Pure data parallel over 8 NeuronCores.
"""

import math
from contextlib import ExitStack

import numpy as np

B_FULL = 8388608
N_CORES = 8
B_SHARD = B_FULL // N_CORES  # 1048576
COLS = B_SHARD // 128        # 8192 cols per partition

NB = 4                       # compute/store blocks of BCOLS cols
BCOLS = COLS // NB           # 2048 cols = 512 KB per f16 block-tensor
DVE_SIN = {3}                # blocks whose sin runs on DVE (SINPOLY)
STORE_RING = ["g", "g", "s", "a"]

MAGIC = 12582912.0           # 1.5 * 2**23: fp32 round-to-nearest-even
TWO_PI = 6.283185307179586
INV_2PI = 1.0 / TWO_PI
# minimax odd deg-7 fit of sin(2*pi*f) on f in [-0.5, 0.5], max err 2.5e-4
SIN_COEF = (6.27864315, -41.09402942, 77.93314409, -56.09367861)

LAST_RESULT = None


def _host_constants(weights: np.ndarray):
    w = np.asarray(weights, dtype=np.float64)

    def rx(t):
        c, s = np.cos(t / 2), np.sin(t / 2)
        return np.array([[c, -1j * s], [-1j * s, c]], dtype=np.complex128)

    def rz(t):
        return np.array(
            [[np.exp(-1j * t / 2), 0], [0, np.exp(1j * t / 2)]], dtype=np.complex128
        )

    U = np.eye(2, dtype=np.complex128)
    for i in range(len(w) // 2):
        U = rz(w[2 * i + 1]) @ rx(w[2 * i]) @ U
    A = 2.0 * abs(U[0, 0]) ** 2 - 1.0
    D = 2.0 * (U[0, 0] * np.conj(U[0, 1])).real
    R = math.hypot(A, D)
    phi = math.atan2(A, D)
    return float(R), float(phi)


# ---- custom DVE ops --------------------------------------------------------


def _register_ops():
    """Define + register the two fused DVE ops (idempotent).  uops_sha is
    self-pinned from lower() so concourse-version drift can't stale-pin;
    numerics are verified on HW by the caller's reference check."""
    import concourse.dve_ops as dmod
    from concourse.dve_ops import DveOp
    from concourse.dve_spec import (
        C0, C1, C2, C3, Spec, Src0, Src1, _has_src1, _spill_c3_to_src1, lower, sq,
    )
    from concourse.dve_uop import DveOpSpec

    have = {op.name: op for op in dmod.OPS}
    if "QNN_SUMWRAP" in have:
        return have["QNN_SUMWRAP"], have["QNN_SINPOLY"]

    # f = t2 - rne(t2); t2 = (x0 + x1)*C0 + C1;  rne via +/- C2 (=MAGIC)
    u = Src0 + Src1
    t2 = u * C0 + C1
    k = (t2 + C2) - C2
    f = t2 - k

    def ref_sumwrap(in0, in1, s0, s1, imm2):
        u = (in0.astype(np.float32) + in1.astype(np.float32)).astype(np.float32)
        t2 = ((u * np.float32(s0)).astype(np.float32) + np.float32(s1)).astype(
            np.float32
        )
        m = (t2 + np.float32(imm2)).astype(np.float32)
        k = (m - np.float32(imm2)).astype(np.float32)
        return (t2 - k).astype(np.float32)

    # y = f*(c1 + z*(c3 + z*(c5 + z*c7))), z=f^2; c7 rides the C3 slot,
    # spilled to a latched [P,1] in1 read (Src1 as a full stream with a
    # [P,1] AP wedges the DVE -- the stream runs dry after one element)
    z = sq(Src0)
    t = ((z * C3 + C0) * z + C1) * z + C2
    y = _spill_c3_to_src1(t * Src0)

    def ref_sinpoly(in0, in1, s0, s1, imm2):
        x = in0.astype(np.float32)
        z = (x * x).astype(np.float32)
        c7 = np.asarray(in1, np.float32).reshape(-1, 1)
        t = (z * c7).astype(np.float32)
        t = (t + np.float32(s0)).astype(np.float32)
        t = (t * z).astype(np.float32)
        t = (t + np.float32(s1)).astype(np.float32)
        t = (t * z).astype(np.float32)
        t = (t + np.float32(imm2)).astype(np.float32)
        return (t * x).astype(np.float32)

    ops = []
    for name, spec in (
        ("QNN_SUMWRAP", Spec(body=f, reference=ref_sumwrap)),
        ("QNN_SINPOLY", Spec(body=y, reference=ref_sinpoly)),
    ):
        shas = {}
        for ver in ("v3", "v4"):
            s = DveOpSpec(
                name=name, opcode=None, uops=lower(spec, ver=ver),
                rd1_en=_has_src1(spec),
            )
            shas[ver] = s.sha(ver)
        op = DveOp(name, spec, subdim=False, uops_sha=shas)
        row = dmod._CUSTOM_DVE_ROW_BASE + len(dmod.OPS)
        assert row < 0x20
        dmod.OPS.append(op)
        dmod._SUB_OPCODE_FOR_NAME[op.name] = row
        dmod.CUSTOM_DVE_SPECS[op.name] = op.spec
        ops.append(op)
    return ops[0], ops[1]


# ---- plan / scheduling -----------------------------------------------------


def _plan_waits(plan):
    """Assign per-op semaphore waits for every RAW/WAR/WAW hazard."""
    semval = {}
    writer = {}
    readers = {}
    seen = {}
    for op in plan:
        want = {}
        for b in op["reads"]:
            if b in writer:
                s, v = writer[b]
                want[s] = max(want.get(s, 0), v)
        for b in op["writes"]:
            for s, v in readers.get(b, []):
                want[s] = max(want.get(s, 0), v)
            if b in writer:
                s, v = writer[b]
                want[s] = max(want.get(s, 0), v)
        eng_seen = seen.setdefault(op["eng"], {})
        waits = []
        for s, v in want.items():
            if eng_seen.get(s, -1) < v:
                waits.append((s, v))
                eng_seen[s] = v
        op["waits"] = waits
        semval[op["sem"]] = semval.get(op["sem"], 0) + op["inc"]
        point = (op["sem"], semval[op["sem"]])
        for b in op["writes"]:
            writer[b] = point
            readers[b] = []
        for b in op["reads"]:
            readers.setdefault(b, []).append(point)
    return plan


def _build_nc(R: float, phi: float):
    import concourse.bacc as bacc
    from concourse import mybir

    SUMWRAP, SINPOLY = _register_ops()

    f16 = mybir.dt.float16
    f32 = mybir.dt.float32
    Sin = mybir.ActivationFunctionType.Sin

    c1, c3, c5, c7 = (c * R for c in SIN_COEF)

    nc = bacc.Bacc(
        "TRN2",
        target_bir_lowering=False,
        debug=False,
        enable_asserts=False,
        num_devices=N_CORES,
    )
    x0 = nc.dram_tensor("x0", [B_SHARD], f16, kind="ExternalInput").ap()
    x1 = nc.dram_tensor("x1", [B_SHARD], f16, kind="ExternalInput").ap()
    y = nc.dram_tensor("y", [B_SHARD], f16, kind="ExternalOutput").ap()
    x0v = x0.rearrange("(p c) -> p c", p=128)
    x1v = x1.rearrange("(p c) -> p c", p=128)
    yv = y.rearrange("(p c) -> p c", p=128)

    x0t = nc.alloc_sbuf_tensor("x0t", [128, COLS], f16)
    x1t = nc.alloc_sbuf_tensor("x1t", [128, COLS], f16)
    ft = nc.alloc_sbuf_tensor("ft", [128, COLS], f32)
    st = nc.alloc_sbuf_tensor("st", [128, COLS], f16)
    yt = nc.alloc_sbuf_tensor("yt", [128, COLS], f16)
    c7t = nc.alloc_sbuf_tensor("c7t", [128, 1], f32)

    def csl(b):
        return slice(b * BCOLS, (b + 1) * BCOLS)

    # ---- phase 1: global plan -------------------------------------------
    def op(eng, kind, i, reads, writes, sem, inc=1):
        return dict(eng=eng, kind=kind, i=i, reads=reads, writes=writes,
                    sem=sem, inc=inc)

    plan = []
    for b in range(NB):  # loads: x0 chunks on sync ring, x1 on act ring
        plan.append(op("s", "ld0", b, [], [f"x0_{b}"], f"l0{b}", 16))
        plan.append(op("a", "ld1", b, [], [f"x1_{b}"], f"l1{b}", 16))
    plan.append(op("v", "memset", 0, [], ["c7t"], "vq"))
    for b in range(NB):
        plan.append(op("v", "sumwrap", b, [f"x0_{b}", f"x1_{b}"], [f"f{b}"], "vq"))
        if b in DVE_SIN:
            plan.append(op("v", "sinpoly", b, [f"f{b}", "c7t"], [f"y{b}"], "vq"))
    # ACT stream: producer->consumer never adjacent (deep pipeline drain)
    act_blocks = [b for b in range(NB) if b not in DVE_SIN]
    acts = [("sin", b) for b in act_blocks]
    pend = []
    sched = []
    for kind, b in acts:
        sched.append((kind, b))
        pend.append(("mul", b))
        if len(sched) >= 2 and pend:
            sched.append(pend.pop(0))
    sched.extend(pend)
    for kind, b in sched:
        if kind == "sin":
            plan.append(op("a", "sin", b, [f"f{b}"], [f"s{b}"], "aq"))
        else:
            plan.append(op("a", "mul", b, [f"s{b}"], [f"y{b}"], "aq"))
    for b in range(NB):
        plan.append(op(STORE_RING[b], "store", b, [f"y{b}"], [], f"os{b}", 16))

    _plan_waits(plan)

    # ---- phase 2: emit per-engine streams -------------------------------
    with ExitStack() as ctx:
        sems = {}
        for o in plan:
            if o["sem"] not in sems:
                sems[o["sem"]] = ctx.enter_context(nc.semaphore(o["sem"]))
        block = ctx.enter_context(nc.Block())

        def emit(o, eng):
            for s, v in o["waits"]:
                eng.wait_ge(sems[s], v)
            b = o["i"]
            k = o["kind"]
            c = csl(b)
            if k == "ld0":
                inst = eng.dma_start(x0t.ap()[:, c], x0v[:, c])
            elif k == "ld1":
                inst = eng.dma_start(x1t.ap()[:, c], x1v[:, c])
            elif k == "store":
                inst = eng.dma_start(yv[:, c], yt.ap()[:, c])
            elif k == "memset":
                inst = nc.vector.memset(c7t.ap(), c7)
            elif k == "sumwrap":
                inst = nc.vector._custom_dve(
                    SUMWRAP, out=ft.ap()[:, c], in0=x0t.ap()[:, c],
                    in1=x1t.ap()[:, c], s0=INV_2PI, s1=phi * INV_2PI,
                    imm2=MAGIC,
                )
            elif k == "sinpoly":
                inst = nc.vector._custom_dve(
                    SINPOLY, out=yt.ap()[:, c], in0=ft.ap()[:, c],
                    in1=c7t.ap(), s0=c5, s1=c3, imm2=c1,
                )
            elif k == "sin":
                inst = nc.scalar.activation(
                    st.ap()[:, c], ft.ap()[:, c], Sin, bias=0.0, scale=TWO_PI
                )
            elif k == "mul":
                inst = nc.scalar.mul(yt.ap()[:, c], st.ap()[:, c], R)
            else:
                raise AssertionError(k)
            inst.then_inc(sems[o["sem"]], o["inc"])

        @block.sync
        def _(sync):
            for o in plan:
                if o["eng"] == "s":
                    emit(o, sync)
            for b in range(NB):
                if STORE_RING[b] == "s":
                    sync.wait_ge(sems[f"os{b}"], 16)

        @block.vector
        def _(vector):
            for o in plan:
                if o["eng"] == "v":
                    emit(o, vector)

        @block.gpsimd
        def _(gpsimd):
            for o in plan:
                if o["eng"] == "g":
                    emit(o, gpsimd)
            for b in range(NB):
                if STORE_RING[b] == "g":
                    gpsimd.wait_ge(sems[f"os{b}"], 16)

        @block.scalar
        def _(scalar):
            for o in plan:
                if o["eng"] == "a":
                    emit(o, scalar)
            for b in range(NB):
                if STORE_RING[b] == "a":
                    scalar.wait_ge(sems[f"os{b}"], 16)

    nc.compile()
    return nc


def kernel(inputs: np.ndarray, weights: np.ndarray, _trace: bool = False) -> np.ndarray:
    global LAST_RESULT
    from concourse.bass_utils import run_bass_kernel_spmd

    inputs = np.asarray(inputs, dtype=np.float32)
    assert inputs.shape == (B_FULL, 2), inputs.shape

    R, phi = _host_constants(weights)
    nc = _build_nc(R, phi)

    xh = inputs.astype(np.float16)
    x0 = np.ascontiguousarray(xh[:, 0])
    x1 = np.ascontiguousarray(xh[:, 1])
    in_maps = [
        {
            "x0": x0[c * B_SHARD : (c + 1) * B_SHARD],
            "x1": x1[c * B_SHARD : (c + 1) * B_SHARD],
        }
        for c in range(N_CORES)
    ]
    res = run_bass_kernel_spmd(
        nc, in_maps, core_ids=list(range(N_CORES)), trace=_trace
    )
    LAST_RESULT = res
    out = np.concatenate([r["y"] for r in res.results], axis=0)
    return out.astype(np.float32).reshape(B_FULL, 1)


# revision 6
# speedup vs baseline: 1.4258x; 1.0209x over previous
"""Trainium2 Bass kernel for nn_EstimatorQNN.

Math reduction: the reference applies a batch-independent 2x2 unitary U
(built from the 4 weights) to |psi> = [cos(th/2), sin(th/2)] with
th = x0 + x1, then returns |amp0|^2 - |amp1|^2.  By unitarity this
collapses to

    out = R*sin(th + phi)

with R = hypot(A, D), phi = atan2(A, D), A = 2|U00|^2 - 1,
D = 2*Re(U00*conj(U01)) -- 2 scalars computed on host from the weights;
the device does the memory-bound elementwise part.

HBM traffic is halved vs f32 by staging the inputs as de-interleaved
fp16 streams and storing fp16 (host widens to f32; tolerance is 2e-2,
fp16 costs ~5e-4).  Per core: 4 MB loads + 2 MB stores = 6 MB.

Compute is two fused custom-DVE ops (registered via the documented
dve_ops.OPS extension point; shas self-pinned at import):

  QNN_SUMWRAP:  f = t2 - rne_round(t2),  t2 = (x0+x1)*(1/2pi) + phi/2pi
                (round via the fp32 +/-1.5*2^23 magic trick).  f is the
                angle in *turns*, exactly in [-0.5, 0.5] for all inputs
                -- no range-reduction tail cases.  1 elem/lane/cycle.
  QNN_SINPOLY:  y = R*sin(2pi*f) as an odd minimax degree-7 polynomial
                (R folded into the coefficients, max err 2.5e-4); c7
                rides the C3 slot latched from a [P,1] in1.

Per column-block: DVE runs SUMWRAP; sin is either ACT's table (free
input scale=2pi) or DVE's SINPOLY; the R-mul runs on ACT (Copy) or DVE
(tensor_scalar fp16).  ACT ops are ordered all-sins-then-all-muls: each
Sin<->Copy function switch reloads the ACT table (1.28 us).  Raw-Bass
hand-scheduled: loads on the two HWDGE rings with a small first chunk
(early first compute) and small last chunk (short tail); stores on
gpsimd/sync/act rings; explicit semaphore waits for every hazard.
Pure data parallel over 8 NeuronCores.
"""

import math
from contextlib import ExitStack

import numpy as np

B_FULL = 8388608
N_CORES = 8
B_SHARD = B_FULL // N_CORES  # 1048576
COLS = B_SHARD // 128        # 8192 cols per partition

# Block layout: LOAD_COLS[i] is both the i-th load-chunk and the i-th
# compute/store block (aligned 1:1).  SIN_DVE blocks evaluate sin via
# QNN_SINPOLY on DVE (no separate mul); others use ACT Sin + a mul on
# ACT (Copy) or DVE (MUL_DVE).
CONFIG = dict(
    load_cols=[1024, 1024, 2048, 2048, 2048],
    sin_dve={4},
    mul_dve={3},
    store_ring=["g", "g", "s", "s", "a"],
)

MAGIC = 12582912.0           # 1.5 * 2**23: fp32 round-to-nearest-even
TWO_PI = 6.283185307179586
INV_2PI = 1.0 / TWO_PI
# minimax odd deg-7 fit of sin(2*pi*f) on f in [-0.5, 0.5], max err 2.5e-4
SIN_COEF = (6.27864315, -41.09402942, 77.93314409, -56.09367861)

LAST_RESULT = None


def _host_constants(weights: np.ndarray):
    w = np.asarray(weights, dtype=np.float64)

    def rx(t):
        c, s = np.cos(t / 2), np.sin(t / 2)
        return np.array([[c, -1j * s], [-1j * s, c]], dtype=np.complex128)

    def rz(t):
        return np.array(
            [[np.exp(-1j * t / 2), 0], [0, np.exp(1j * t / 2)]], dtype=np.complex128
        )

    U = np.eye(2, dtype=np.complex128)
    for i in range(len(w) // 2):
        U = rz(w[2 * i + 1]) @ rx(w[2 * i]) @ U
    A = 2.0 * abs(U[0, 0]) ** 2 - 1.0
    D = 2.0 * (U[0, 0] * np.conj(U[0, 1])).real
    R = math.hypot(A, D)
    phi = math.atan2(A, D)
    return float(R), float(phi)


# ---- custom DVE ops --------------------------------------------------------


def _register_ops():
    """Define + register the two fused DVE ops (idempotent).  uops_sha is
    self-pinned from lower() so concourse-version drift can't stale-pin;
    numerics are verified against the reference by the test harness."""
    import concourse.dve_ops as dmod
    from concourse.dve_ops import DveOp
    from concourse.dve_spec import (
        C0, C1, C2, C3, Spec, Src0, _has_src1, _spill_c3_to_src1, lower, sq,
    )
    from concourse.dve_spec import Src1  # noqa: F401  (spill target)
    from concourse.dve_uop import DveOpSpec

    have = {op.name: op for op in dmod.OPS}
    if "QNN_SUMWRAP" in have:
        return have["QNN_SUMWRAP"], have["QNN_SINPOLY"]

    # f = t2 - rne(t2); t2 = (x0 + x1)*C0 + C1;  rne via +/- C2 (=MAGIC)
    from concourse.dve_spec import Src1 as S1

    u = Src0 + S1
    t2 = u * C0 + C1
    k = (t2 + C2) - C2
    f = t2 - k

    def ref_sumwrap(in0, in1, s0, s1, imm2):
        u = (in0.astype(np.float32) + in1.astype(np.float32)).astype(np.float32)
        t2 = ((u * np.float32(s0)).astype(np.float32) + np.float32(s1)).astype(
            np.float32
        )
        m = (t2 + np.float32(imm2)).astype(np.float32)
        k = (m - np.float32(imm2)).astype(np.float32)
        return (t2 - k).astype(np.float32)

    # y = f*(c1 + z*(c3 + z*(c5 + z*c7))), z=f^2; c7 rides the C3 slot,
    # spilled to a latched [P,1] in1 read (Src1 as a full stream with a
    # [P,1] AP wedges the DVE -- the stream runs dry after one element)
    z = sq(Src0)
    t = ((z * C3 + C0) * z + C1) * z + C2
    y = _spill_c3_to_src1(t * Src0)

    def ref_sinpoly(in0, in1, s0, s1, imm2):
        x = in0.astype(np.float32)
        z = (x * x).astype(np.float32)
        c7 = np.asarray(in1, np.float32).reshape(-1, 1)
        t = (z * c7).astype(np.float32)
        t = (t + np.float32(s0)).astype(np.float32)
        t = (t * z).astype(np.float32)
        t = (t + np.float32(s1)).astype(np.float32)
        t = (t * z).astype(np.float32)
        t = (t + np.float32(imm2)).astype(np.float32)
        return (t * x).astype(np.float32)

    ops = []
    for name, spec in (
        ("QNN_SUMWRAP", Spec(body=f, reference=ref_sumwrap)),
        ("QNN_SINPOLY", Spec(body=y, reference=ref_sinpoly)),
    ):
        shas = {}
        for ver in ("v3", "v4"):
            s = DveOpSpec(
                name=name, opcode=None, uops=lower(spec, ver=ver),
                rd1_en=_has_src1(spec),
            )
            shas[ver] = s.sha(ver)
        op = DveOp(name, spec, subdim=False, uops_sha=shas)
        row = dmod._CUSTOM_DVE_ROW_BASE + len(dmod.OPS)
        assert row < 0x20
        dmod.OPS.append(op)
        dmod._SUB_OPCODE_FOR_NAME[op.name] = row
        dmod.CUSTOM_DVE_SPECS[op.name] = op.spec
        ops.append(op)
    return ops[0], ops[1]


# ---- plan / scheduling -----------------------------------------------------


def _plan_waits(plan):
    """Assign per-op semaphore waits for every RAW/WAR/WAW hazard."""
    semval = {}
    writer = {}
    readers = {}
    seen = {}
    for op in plan:
        want = {}
        for b in op["reads"]:
            if b in writer:
                s, v = writer[b]
                want[s] = max(want.get(s, 0), v)
        for b in op["writes"]:
            for s, v in readers.get(b, []):
                want[s] = max(want.get(s, 0), v)
            if b in writer:
                s, v = writer[b]
                want[s] = max(want.get(s, 0), v)
        eng_seen = seen.setdefault(op["eng"], {})
        waits = []
        for s, v in want.items():
            if eng_seen.get(s, -1) < v:
                waits.append((s, v))
                eng_seen[s] = v
        op["waits"] = waits
        semval[op["sem"]] = semval.get(op["sem"], 0) + op["inc"]
        point = (op["sem"], semval[op["sem"]])
        for b in op["writes"]:
            writer[b] = point
            readers[b] = []
        for b in op["reads"]:
            readers.setdefault(b, []).append(point)
    return plan


def _build_nc(R: float, phi: float, cfg=None):
    import concourse.bacc as bacc
    from concourse import mybir

    cfg = cfg or CONFIG
    LOAD_COLS = cfg["load_cols"]
    SIN_DVE = cfg["sin_dve"]
    MUL_DVE = cfg["mul_dve"]
    STORE_RING = cfg["store_ring"]
    NB = len(LOAD_COLS)
    assert sum(LOAD_COLS) == COLS
    off = [sum(LOAD_COLS[:i]) for i in range(NB)]

    SUMWRAP, SINPOLY = _register_ops()

    f16 = mybir.dt.float16
    f32 = mybir.dt.float32
    Sin = mybir.ActivationFunctionType.Sin

    c1, c3, c5, c7 = (c * R for c in SIN_COEF)

    nc = bacc.Bacc(
        "TRN2",
        target_bir_lowering=False,
        debug=False,
        enable_asserts=False,
        num_devices=N_CORES,
    )
    x0 = nc.dram_tensor("x0", [B_SHARD], f16, kind="ExternalInput").ap()
    x1 = nc.dram_tensor("x1", [B_SHARD], f16, kind="ExternalInput").ap()
    y = nc.dram_tensor("y", [B_SHARD], f16, kind="ExternalOutput").ap()
    x0v = x0.rearrange("(p c) -> p c", p=128)
    x1v = x1.rearrange("(p c) -> p c", p=128)
    yv = y.rearrange("(p c) -> p c", p=128)

    x0t = nc.alloc_sbuf_tensor("x0t", [128, COLS], f16)
    x1t = nc.alloc_sbuf_tensor("x1t", [128, COLS], f16)
    ft = nc.alloc_sbuf_tensor("ft", [128, COLS], f32)
    st = nc.alloc_sbuf_tensor("st", [128, COLS], f16)
    yt = nc.alloc_sbuf_tensor("yt", [128, COLS], f16)
    c7t = nc.alloc_sbuf_tensor("c7t", [128, 1], f32)

    def csl(b):
        return slice(off[b], off[b] + LOAD_COLS[b])

    # ---- phase 1: global plan -------------------------------------------
    def op(eng, kind, i, reads, writes, sem, inc=1):
        return dict(eng=eng, kind=kind, i=i, reads=reads, writes=writes,
                    sem=sem, inc=inc)

    plan = []
    for b in range(NB):  # loads: x0 chunks on sync ring, x1 on act ring
        plan.append(op("s", "ld0", b, [], [f"x0_{b}"], f"l0{b}", 16))
        plan.append(op("a", "ld1", b, [], [f"x1_{b}"], f"l1{b}", 16))
    plan.append(op("v", "memset", 0, [], ["c7t"], "vq"))
    # NOTE: the plan list must be a valid topological execution order --
    # _plan_waits resolves hazards by linear scan.  Engine streams are
    # the per-engine subsequences of this order.
    for b in range(NB):
        plan.append(op("v", "sumwrap", b, [f"x0_{b}", f"x1_{b}"], [f"f{b}"], "vq"))
        if b in SIN_DVE:  # right after its producer on the same engine
            plan.append(op("v", "sinpoly", b, [f"f{b}", "c7t"], [f"y{b}"], "vq"))
    # ACT stream: all sins in block order, then all muls (one table switch)
    for b in range(NB):
        if b not in SIN_DVE:
            plan.append(op("a", "sin", b, [f"f{b}"], [f"s{b}"], "aq"))
    for b in range(NB):
        if b not in SIN_DVE and b in MUL_DVE:
            plan.append(op("v", "dmul", b, [f"s{b}"], [f"y{b}"], "vq"))
    for b in range(NB):
        if b not in SIN_DVE and b not in MUL_DVE:
            plan.append(op("a", "mul", b, [f"s{b}"], [f"y{b}"], "aq"))
    for b in range(NB):
        plan.append(op(STORE_RING[b], "store", b, [f"y{b}"], [], f"os{b}", 16))

    _plan_waits(plan)

    # ---- phase 2: emit per-engine streams -------------------------------
    with ExitStack() as ctx:
        sems = {}
        for o in plan:
            if o["sem"] not in sems:
                sems[o["sem"]] = ctx.enter_context(nc.semaphore(o["sem"]))
        block = ctx.enter_context(nc.Block())

        def emit(o, eng):
            for s, v in o["waits"]:
                eng.wait_ge(sems[s], v)
            b = o["i"]
            k = o["kind"]
            c = csl(b)
            if k == "ld0":
                inst = eng.dma_start(x0t.ap()[:, c], x0v[:, c])
            elif k == "ld1":
                inst = eng.dma_start(x1t.ap()[:, c], x1v[:, c])
            elif k == "store":
                inst = eng.dma_start(yv[:, c], yt.ap()[:, c])
            elif k == "memset":
                inst = nc.vector.memset(c7t.ap(), c7)
            elif k == "sumwrap":
                inst = nc.vector._custom_dve(
                    SUMWRAP, out=ft.ap()[:, c], in0=x0t.ap()[:, c],
                    in1=x1t.ap()[:, c], s0=INV_2PI, s1=phi * INV_2PI,
                    imm2=MAGIC,
                )
            elif k == "sinpoly":
                inst = nc.vector._custom_dve(
                    SINPOLY, out=yt.ap()[:, c], in0=ft.ap()[:, c],
                    in1=c7t.ap(), s0=c5, s1=c3, imm2=c1,
                )
            elif k == "dmul":
                inst = nc.vector.tensor_scalar_mul(
                    yt.ap()[:, c], st.ap()[:, c], R
                )
            elif k == "sin":
                inst = nc.scalar.activation(
                    st.ap()[:, c], ft.ap()[:, c], Sin, bias=0.0, scale=TWO_PI
                )
            elif k == "mul":
                inst = nc.scalar.mul(yt.ap()[:, c], st.ap()[:, c], R)
            else:
                raise AssertionError(k)
            inst.then_inc(sems[o["sem"]], o["inc"])

        @block.sync
        def _(sync):
            for o in plan:
                if o["eng"] == "s":
                    emit(o, sync)
            for b in range(NB):
                if STORE_RING[b] == "s":
                    sync.wait_ge(sems[f"os{b}"], 16)

        @block.vector
        def _(vector):
            for o in plan:
                if o["eng"] == "v":
                    emit(o, vector)

        @block.gpsimd
        def _(gpsimd):
            for o in plan:
                if o["eng"] == "g":
                    emit(o, gpsimd)
            for b in range(NB):
                if STORE_RING[b] == "g":
                    gpsimd.wait_ge(sems[f"os{b}"], 16)

        @block.scalar
        def _(scalar):
            for o in plan:
                if o["eng"] == "a":
                    emit(o, scalar)
            for b in range(NB):
                if STORE_RING[b] == "a":
                    scalar.wait_ge(sems[f"os{b}"], 16)

    nc.compile()
    return nc


def kernel(inputs: np.ndarray, weights: np.ndarray, _trace: bool = False,
           _cfg=None) -> np.ndarray:
    global LAST_RESULT
    from concourse.bass_utils import run_bass_kernel_spmd

    inputs = np.asarray(inputs, dtype=np.float32)
    assert inputs.shape == (B_FULL, 2), inputs.shape

    R, phi = _host_constants(weights)
    nc = _build_nc(R, phi, _cfg)

    xh = inputs.astype(np.float16)
    x0 = np.ascontiguousarray(xh[:, 0])
    x1 = np.ascontiguousarray(xh[:, 1])
    in_maps = [
        {
            "x0": x0[c * B_SHARD : (c + 1) * B_SHARD],
            "x1": x1[c * B_SHARD : (c + 1) * B_SHARD],
        }
        for c in range(N_CORES)
    ]
    res = run_bass_kernel_spmd(
        nc, in_maps, core_ids=list(range(N_CORES)), trace=_trace
    )
    LAST_RESULT = res
    out = np.concatenate([r["y"] for r in res.results], axis=0)
    return out.astype(np.float32).reshape(B_FULL, 1)


# revision 20
# speedup vs baseline: 1.5349x; 1.0765x over previous
"""Trainium2 Bass kernel for nn_EstimatorQNN.

Math reduction: the reference applies a batch-independent 2x2 unitary U
(built from the 4 weights) to |psi> = [cos(th/2), sin(th/2)] with
th = x0 + x1, then returns |amp0|^2 - |amp1|^2.  By unitarity this
collapses to

    out = R*sin(th + phi)

with R = hypot(A, D), phi = atan2(A, D), A = 2|U00|^2 - 1,
D = 2*Re(U00*conj(U01)) -- 2 scalars computed on host from the weights;
the device does the memory-bound elementwise part.

HBM traffic is halved vs f32 by staging the inputs as de-interleaved
fp16 streams and storing fp16 (host widens to f32; tolerance is 2e-2,
fp16 costs ~5e-4).  Per core: 4 MB loads + 2 MB stores = 6 MB.

Compute is two fused custom-DVE ops (registered via the documented
dve_ops.OPS extension point; shas self-pinned at import):

  QNN_SUMWRAP:  f = t2 - rne_round(t2),  t2 = (x0+x1)*(1/2pi) + phi/2pi
                (round via the fp32 +/-1.5*2^23 magic trick).  f is the
                angle in *turns*, exactly in [-0.5, 0.5] for all inputs
                -- no range-reduction tail cases.  1 elem/lane/cycle.
  QNN_SINPOLY:  y = R*sin(2pi*f) as an odd minimax degree-7 polynomial
                (R folded into the coefficients, max err 2.5e-4); c7
                rides the C3 slot latched from a [P,1] in1.

Per column-block: DVE runs SUMWRAP; sin is either ACT's table (free
input scale=2pi) or DVE's SINPOLY; the R-mul runs on ACT (Copy) or DVE
(tensor_scalar fp16).  ACT ops are ordered all-sins-then-all-muls: each
Sin<->Copy function switch reloads the ACT table (1.28 us).  Raw-Bass
hand-scheduled: loads on the two HWDGE rings with a small first chunk
(early first compute) and small last chunk (short tail); stores on
gpsimd/sync/act rings; explicit semaphore waits for every hazard.
Pure data parallel over 8 NeuronCores.
"""

import math
from contextlib import ExitStack

import numpy as np

B_FULL = 8388608
N_CORES = 8
B_SHARD = B_FULL // N_CORES  # 1048576
COLS = B_SHARD // 128        # 8192 cols per partition

# Block layout: LOAD_COLS[i] is both the i-th load-chunk and the i-th
# compute/store block (aligned 1:1).  SIN_DVE blocks evaluate sin via
# QNN_SINPOLY on DVE (no separate mul); others use ACT Sin + a mul on
# ACT (Copy) or DVE (MUL_DVE).
# Inputs are host-packed per chunk: [x0_b rows | x1_b rows] so each
# chunk is ONE [128, 2*cols] DMA with 2*cols*2B contiguous per-partition
# runs (4KB descriptors at 1024 cols -- ~25% more engine bandwidth than
# the 2KB ones two separate x0/x1 loads would give).
CONFIG = dict(
    load_cols=[512, 512, 1024, 1024, 1024, 1024, 1024, 1024, 512, 512],
    load_ring="sasasasasa",      # per-chunk dispatch ring
    sin_dve={8, 9},              # tail blocks: shortest path to the store
    mul_dve={0, 1, 2, 3, 4, 5, 6, 7},
    store_group=[(0, 1), (2, 3), (4, 5), (6, 7), (8,), (9,)],
    store_ring=["g", "s", "g", "s", "g", "s"],
    dmul_dist=1,
)

MAGIC = 12582912.0           # 1.5 * 2**23: fp32 round-to-nearest-even
TWO_PI = 6.283185307179586
INV_2PI = 1.0 / TWO_PI
# minimax odd deg-7 fit of sin(2*pi*f) on f in [-0.5, 0.5], max err 2.5e-4
SIN_COEF = (6.27864315, -41.09402942, 77.93314409, -56.09367861)

LAST_RESULT = None


def _host_constants(weights: np.ndarray):
    w = np.asarray(weights, dtype=np.float64)

    def rx(t):
        c, s = np.cos(t / 2), np.sin(t / 2)
        return np.array([[c, -1j * s], [-1j * s, c]], dtype=np.complex128)

    def rz(t):
        return np.array(
            [[np.exp(-1j * t / 2), 0], [0, np.exp(1j * t / 2)]], dtype=np.complex128
        )

    U = np.eye(2, dtype=np.complex128)
    for i in range(len(w) // 2):
        U = rz(w[2 * i + 1]) @ rx(w[2 * i]) @ U
    A = 2.0 * abs(U[0, 0]) ** 2 - 1.0
    D = 2.0 * (U[0, 0] * np.conj(U[0, 1])).real
    R = math.hypot(A, D)
    phi = math.atan2(A, D)
    return float(R), float(phi)


# ---- custom DVE ops --------------------------------------------------------


def _register_ops():
    """Define + register the two fused DVE ops (idempotent).  uops_sha is
    self-pinned from lower() so concourse-version drift can't stale-pin;
    numerics are verified against the reference by the test harness."""
    import concourse.dve_ops as dmod
    from concourse.dve_ops import DveOp
    from concourse.dve_spec import (
        C0, C1, C2, C3, Spec, Src0, _has_src1, _spill_c3_to_src1, lower, sq,
    )
    from concourse.dve_spec import Src1  # noqa: F401  (spill target)
    from concourse.dve_uop import DveOpSpec

    have = {op.name: op for op in dmod.OPS}
    if "QNN_SUMWRAP" in have:
        return have["QNN_SUMWRAP"], have["QNN_SINPOLY"]

    # f = t2 - rne(t2); t2 = (x0 + x1)*C0 + C1;  rne via +/- C2 (=MAGIC)
    from concourse.dve_spec import Src1 as S1

    u = Src0 + S1
    t2 = u * C0 + C1
    k = (t2 + C2) - C2
    f = t2 - k

    def ref_sumwrap(in0, in1, s0, s1, imm2):
        u = (in0.astype(np.float32) + in1.astype(np.float32)).astype(np.float32)
        t2 = ((u * np.float32(s0)).astype(np.float32) + np.float32(s1)).astype(
            np.float32
        )
        m = (t2 + np.float32(imm2)).astype(np.float32)
        k = (m - np.float32(imm2)).astype(np.float32)
        return (t2 - k).astype(np.float32)

    # y = f*(c1 + z*(c3 + z*(c5 + z*c7))), z=f^2; c7 rides the C3 slot,
    # spilled to a latched [P,1] in1 read (Src1 as a full stream with a
    # [P,1] AP wedges the DVE -- the stream runs dry after one element)
    z = sq(Src0)
    t = ((z * C3 + C0) * z + C1) * z + C2
    y = _spill_c3_to_src1(t * Src0)

    def ref_sinpoly(in0, in1, s0, s1, imm2):
        x = in0.astype(np.float32)
        z = (x * x).astype(np.float32)
        c7 = np.asarray(in1, np.float32).reshape(-1, 1)
        t = (z * c7).astype(np.float32)
        t = (t + np.float32(s0)).astype(np.float32)
        t = (t * z).astype(np.float32)
        t = (t + np.float32(s1)).astype(np.float32)
        t = (t * z).astype(np.float32)
        t = (t + np.float32(imm2)).astype(np.float32)
        return (t * x).astype(np.float32)

    ops = []
    for name, spec in (
        ("QNN_SUMWRAP", Spec(body=f, reference=ref_sumwrap)),
        ("QNN_SINPOLY", Spec(body=y, reference=ref_sinpoly)),
    ):
        shas = {}
        for ver in ("v3", "v4"):
            s = DveOpSpec(
                name=name, opcode=None, uops=lower(spec, ver=ver),
                rd1_en=_has_src1(spec),
            )
            shas[ver] = s.sha(ver)
        op = DveOp(name, spec, subdim=False, uops_sha=shas)
        row = dmod._CUSTOM_DVE_ROW_BASE + len(dmod.OPS)
        assert row < 0x20
        dmod.OPS.append(op)
        dmod._SUB_OPCODE_FOR_NAME[op.name] = row
        dmod.CUSTOM_DVE_SPECS[op.name] = op.spec
        ops.append(op)
    return ops[0], ops[1]


# ---- plan / scheduling -----------------------------------------------------


def _plan_waits(plan):
    """Assign per-op semaphore waits for every RAW/WAR/WAW hazard."""
    semval = {}
    writer = {}
    readers = {}
    seen = {}
    for op in plan:
        want = {}
        for b in op["reads"]:
            if b in writer:
                s, v = writer[b]
                want[s] = max(want.get(s, 0), v)
        for b in op["writes"]:
            for s, v in readers.get(b, []):
                want[s] = max(want.get(s, 0), v)
            if b in writer:
                s, v = writer[b]
                want[s] = max(want.get(s, 0), v)
        eng_seen = seen.setdefault(op["eng"], {})
        waits = []
        for s, v in want.items():
            if eng_seen.get(s, -1) < v:
                waits.append((s, v))
                eng_seen[s] = v
        op["waits"] = waits
        semval[op["sem"]] = semval.get(op["sem"], 0) + op["inc"]
        point = (op["sem"], semval[op["sem"]])
        for b in op["writes"]:
            writer[b] = point
            readers[b] = []
        for b in op["reads"]:
            readers.setdefault(b, []).append(point)
    return plan


def _build_nc(R: float, phi: float, cfg=None):
    import concourse.bacc as bacc
    from concourse import mybir

    cfg = cfg or CONFIG
    LOAD_COLS = cfg["load_cols"]
    LOAD_RING = cfg["load_ring"]
    SIN_DVE = cfg["sin_dve"]
    MUL_DVE = cfg["mul_dve"]
    STORE_GROUP = cfg["store_group"]
    STORE_RING = cfg["store_ring"]
    NB = len(LOAD_COLS)
    assert sum(LOAD_COLS) == COLS
    off = [sum(LOAD_COLS[:i]) for i in range(NB)]

    SUMWRAP, SINPOLY = _register_ops()

    f16 = mybir.dt.float16
    f32 = mybir.dt.float32
    Sin = mybir.ActivationFunctionType.Sin

    c1, c3, c5, c7 = (c * R for c in SIN_COEF)

    nc = bacc.Bacc(
        "TRN2",
        target_bir_lowering=False,
        debug=False,
        enable_asserts=False,
        num_devices=N_CORES,
    )
    xp = nc.dram_tensor("xp", [2 * B_SHARD], f16, kind="ExternalInput").ap()
    y = nc.dram_tensor("y", [B_SHARD], f16, kind="ExternalOutput").ap()
    xpv = xp.rearrange("(p c) -> p c", p=128)  # [128, 2*COLS], chunk-packed
    yv = y.rearrange("(p c) -> p c", p=128)

    xt = nc.alloc_sbuf_tensor("xt", [128, 2 * COLS], f16)
    ft = nc.alloc_sbuf_tensor("ft", [128, COLS], f32)
    st = nc.alloc_sbuf_tensor("st", [128, COLS], f16)
    yt = nc.alloc_sbuf_tensor("yt", [128, COLS], f16)
    c7t = nc.alloc_sbuf_tensor("c7t", [128, 1], f32)

    def csl(b):
        return slice(off[b], off[b] + LOAD_COLS[b])

    # ---- phase 1: global plan -------------------------------------------
    def op(eng, kind, i, reads, writes, sem, inc=1):
        return dict(eng=eng, kind=kind, i=i, reads=reads, writes=writes,
                    sem=sem, inc=inc)

    MERGE = cfg.get("merge_sems", False)
    plan = []
    for b in range(NB):  # one packed [x0_b | x1_b] load per chunk
        # merged mode: all loads on one ring share one sem -- ring FIFO
        # makes chunk b complete exactly at lq >= 16*(b+1)
        sem = "lq" if MERGE else f"l{b}"
        plan.append(op(LOAD_RING[b], "ld", b, [], [f"x_{b}"], sem, 16))
    plan.append(op("v", "memset", 0, [], ["c7t"], "vq"))
    # NOTE: the plan list must be a valid topological execution order --
    # _plan_waits resolves hazards by linear scan.  Engine streams are
    # the per-engine subsequences of this order.  DVE muls are slotted
    # two sumwraps behind their block so (a) the ACT sin is long done
    # when DVE reaches the mul (no stall) and (b) the mul fills DVE's
    # load-wait gaps, making y blocks -- and their stores -- early.
    DIST = cfg.get("dmul_dist", 2)
    for b in range(NB):
        plan.append(op("v", "sumwrap", b, [f"x_{b}"], [f"f{b}"], "vq"))
        if b in SIN_DVE:  # right after its producer on the same engine
            plan.append(op("v", "sinpoly", b, [f"f{b}", "c7t"], [f"y{b}"], "vq"))
        if b not in SIN_DVE:
            plan.append(op("a", "sin", b, [f"f{b}"], [f"s{b}"], "aq"))
        d = b - DIST
        if d >= 0 and d in MUL_DVE and d not in SIN_DVE:
            plan.append(op("v", "dmul", d, [f"s{d}"], [f"y{d}"], "vq"))
    for d in range(max(0, NB - DIST), NB):
        if d in MUL_DVE and d not in SIN_DVE:
            plan.append(op("v", "dmul", d, [f"s{d}"], [f"y{d}"], "vq"))
    for b in range(NB):
        if b not in SIN_DVE and b not in MUL_DVE:
            plan.append(op("a", "mul", b, [f"s{b}"], [f"y{b}"], "aq"))
    for g, blocks in enumerate(STORE_GROUP):
        sem = f"os_{STORE_RING[g]}" if MERGE else f"os{g}"
        plan.append(op(STORE_RING[g], "store", g,
                       [f"y{b}" for b in blocks], [], sem, 16))

    _plan_waits(plan)

    # ---- phase 2: emit per-engine streams -------------------------------
    with ExitStack() as ctx:
        sems = {}
        for o in plan:
            if o["sem"] not in sems:
                sems[o["sem"]] = ctx.enter_context(nc.semaphore(o["sem"]))
        block = ctx.enter_context(nc.Block())

        def emit(o, eng):
            for s, v in o["waits"]:
                eng.wait_ge(sems[s], v)
            b = o["i"]
            k = o["kind"]
            if k == "ld":
                pc = slice(2 * off[b], 2 * (off[b] + LOAD_COLS[b]))
                inst = eng.dma_start(xt.ap()[:, pc], xpv[:, pc])
            elif k == "store":
                blocks = STORE_GROUP[b]
                c = slice(off[blocks[0]],
                          off[blocks[-1]] + LOAD_COLS[blocks[-1]])
                inst = eng.dma_start(yv[:, c], yt.ap()[:, c])
            elif k == "memset":
                inst = nc.vector.memset(c7t.ap(), c7)
            elif k == "sumwrap":
                c = csl(b)
                lo, cb = 2 * off[b], LOAD_COLS[b]
                inst = nc.vector._custom_dve(
                    SUMWRAP, out=ft.ap()[:, c],
                    in0=xt.ap()[:, lo : lo + cb],
                    in1=xt.ap()[:, lo + cb : lo + 2 * cb],
                    s0=INV_2PI, s1=phi * INV_2PI, imm2=MAGIC,
                )
            elif k == "sinpoly":
                c = csl(b)
                inst = nc.vector._custom_dve(
                    SINPOLY, out=yt.ap()[:, c], in0=ft.ap()[:, c],
                    in1=c7t.ap(), s0=c5, s1=c3, imm2=c1,
                )
            elif k == "dmul":
                c = csl(b)
                inst = nc.vector.tensor_scalar_mul(
                    yt.ap()[:, c], st.ap()[:, c], R
                )
            elif k == "sin":
                c = csl(b)
                inst = nc.scalar.activation(
                    st.ap()[:, c], ft.ap()[:, c], Sin, bias=0.0, scale=TWO_PI
                )
            elif k == "mul":
                c = csl(b)
                inst = nc.scalar.mul(yt.ap()[:, c], st.ap()[:, c], R)
            else:
                raise AssertionError(k)
            inst.then_inc(sems[o["sem"]], o["inc"])

        NG = len(STORE_GROUP)

        def store_waits(eng, ring):
            if MERGE:
                n = sum(1 for r in STORE_RING if r == ring)
                if n:
                    eng.wait_ge(sems[f"os_{ring}"], 16 * n)
                return
            for g in range(NG):
                if STORE_RING[g] == ring:
                    eng.wait_ge(sems[f"os{g}"], 16)

        @block.sync
        def _(sync):
            for o in plan:
                if o["eng"] == "s":
                    emit(o, sync)
            store_waits(sync, "s")

        @block.vector
        def _(vector):
            for o in plan:
                if o["eng"] == "v":
                    emit(o, vector)

        @block.gpsimd
        def _(gpsimd):
            for o in plan:
                if o["eng"] == "g":
                    emit(o, gpsimd)
            store_waits(gpsimd, "g")

        @block.scalar
        def _(scalar):
            for o in plan:
                if o["eng"] == "a":
                    emit(o, scalar)
            store_waits(scalar, "a")

    nc.compile()
    return nc


def kernel(inputs: np.ndarray, weights: np.ndarray, _trace: bool = False,
           _cfg=None) -> np.ndarray:
    global LAST_RESULT
    from concourse.bass_utils import run_bass_kernel_spmd

    inputs = np.asarray(inputs, dtype=np.float32)
    assert inputs.shape == (B_FULL, 2), inputs.shape

    R, phi = _host_constants(weights)
    nc = _build_nc(R, phi, _cfg)

    cfg = _cfg or CONFIG
    load_cols = cfg["load_cols"]
    off = [sum(load_cols[:i]) for i in range(len(load_cols))]
    xh = inputs.astype(np.float16)
    in_maps = []
    for c in range(N_CORES):
        sh = xh[c * B_SHARD : (c + 1) * B_SHARD]
        x0m = sh[:, 0].reshape(128, COLS)
        x1m = sh[:, 1].reshape(128, COLS)
        packed = np.empty((128, 2 * COLS), np.float16)
        for b, cb in enumerate(load_cols):
            o = off[b]
            packed[:, 2 * o : 2 * o + cb] = x0m[:, o : o + cb]
            packed[:, 2 * o + cb : 2 * o + 2 * cb] = x1m[:, o : o + cb]
        in_maps.append({"xp": packed.ravel()})
    res = run_bass_kernel_spmd(
        nc, in_maps, core_ids=list(range(N_CORES)), trace=_trace
    )
    LAST_RESULT = res
    out = np.concatenate([r["y"] for r in res.results], axis=0)
    return out.astype(np.float32).reshape(B_FULL, 1)


# revision 22
# speedup vs baseline: 1.7443x; 1.1365x over previous
"""Trainium2 Bass kernel for nn_EstimatorQNN.

Math reduction: the reference applies a batch-independent 2x2 unitary U
(built from the 4 weights) to |psi> = [cos(th/2), sin(th/2)] with
th = x0 + x1, then returns |amp0|^2 - |amp1|^2.  By unitarity this
collapses to

    out = R*sin(th + phi)

with R = hypot(A, D), phi = atan2(A, D), A = 2|U00|^2 - 1,
D = 2*Re(U00*conj(U01)) -- 2 scalars computed on host from the weights;
the device does the memory-bound elementwise part.

HBM traffic is halved vs f32 by staging the inputs as de-interleaved
fp16 streams and storing fp16 (host widens to f32; tolerance is 2e-2,
fp16 costs ~5e-4).  Per core: 4 MB loads + 2 MB stores = 6 MB.

Compute is two fused custom-DVE ops (registered via the documented
dve_ops.OPS extension point; shas self-pinned at import):

  QNN_SUMWRAP:  f = t2 - rne_round(t2),  t2 = (x0+x1)*(1/2pi) + phi/2pi
                (round via the fp32 +/-1.5*2^23 magic trick).  f is the
                angle in *turns*, exactly in [-0.5, 0.5] for all inputs
                -- no range-reduction tail cases.  1 elem/lane/cycle.
  QNN_SINPOLY:  y = R*sin(2pi*f) as an odd minimax degree-7 polynomial
                (R folded into the coefficients, max err 2.5e-4); c7
                rides the C3 slot latched from a [P,1] in1.

Per column-block: DVE runs SUMWRAP; ACT evaluates sin via its table
(free input scale=2pi) and DVE applies the R-mul as a stock fp16
tensor_scalar (~3x faster than an ACT Copy), slotted one sumwrap
behind its block so it fills DVE's load-wait gaps and the y blocks --
and their stores -- land early enough to overlap the load phase.  The
two tail blocks use DVE's SINPOLY instead (shortest chunk->store
chain, no ACT round trip).  Raw-Bass hand-scheduled: loads alternate
between the two HWDGE rings with small first chunks (early first
compute) and small last chunks (short tail); paired stores on the
gpsimd/sync rings; explicit semaphore waits for every hazard.  The
steady state runs at the per-core HBM roofline (~350 GB/s): chunk
cadence ~1.45 us per 512 KB.  Pure data parallel over 8 NeuronCores.
"""

import math
from contextlib import ExitStack

import numpy as np

B_FULL = 8388608
N_CORES = 8
B_SHARD = B_FULL // N_CORES  # 1048576
COLS = B_SHARD // 128        # 8192 cols per partition

# Block layout: LOAD_COLS[i] is both the i-th load-chunk and the i-th
# compute/store block (aligned 1:1).  SIN_DVE blocks evaluate sin via
# QNN_SINPOLY on DVE (no separate mul); others use ACT Sin + a mul on
# ACT (Copy) or DVE (MUL_DVE).
# Inputs are host-packed per chunk: [x0_b rows | x1_b rows] so each
# chunk is ONE [128, 2*cols] DMA with 2*cols*2B contiguous per-partition
# runs (4KB descriptors at 1024 cols -- ~25% more engine bandwidth than
# the 2KB ones two separate x0/x1 loads would give).
CONFIG = dict(
    load_cols=[512, 512, 1024, 1024, 1024, 1024, 1024, 1024, 512, 512],
    load_ring="sasasasasa",      # per-chunk dispatch ring
    sin_dve={8, 9},              # tail blocks: shortest path to the store
    mul_dve={0, 1, 2, 3, 4, 5, 6, 7},
    store_group=[(0, 1), (2, 3), (4, 5), (6, 7), (8,), (9,)],
    store_ring=["g", "s", "g", "s", "g", "s"],
    dmul_dist=1,
)

MAGIC = 12582912.0           # 1.5 * 2**23: fp32 round-to-nearest-even
TWO_PI = 6.283185307179586
INV_2PI = 1.0 / TWO_PI
# minimax odd deg-7 fit of sin(2*pi*f) on f in [-0.5, 0.5], max err 2.5e-4
SIN_COEF = (6.27864315, -41.09402942, 77.93314409, -56.09367861)

LAST_RESULT = None


def _host_constants(weights: np.ndarray):
    w = np.asarray(weights, dtype=np.float64)

    def rx(t):
        c, s = np.cos(t / 2), np.sin(t / 2)
        return np.array([[c, -1j * s], [-1j * s, c]], dtype=np.complex128)

    def rz(t):
        return np.array(
            [[np.exp(-1j * t / 2), 0], [0, np.exp(1j * t / 2)]], dtype=np.complex128
        )

    U = np.eye(2, dtype=np.complex128)
    for i in range(len(w) // 2):
        U = rz(w[2 * i + 1]) @ rx(w[2 * i]) @ U
    A = 2.0 * abs(U[0, 0]) ** 2 - 1.0
    D = 2.0 * (U[0, 0] * np.conj(U[0, 1])).real
    R = math.hypot(A, D)
    phi = math.atan2(A, D)
    return float(R), float(phi)


# ---- custom DVE ops --------------------------------------------------------


def _register_ops():
    """Define + register the two fused DVE ops (idempotent).  uops_sha is
    self-pinned from lower() so concourse-version drift can't stale-pin;
    numerics are verified against the reference by the test harness."""
    import concourse.dve_ops as dmod
    from concourse.dve_ops import DveOp
    from concourse.dve_spec import (
        C0, C1, C2, C3, Spec, Src0, _has_src1, _spill_c3_to_src1, lower, sq,
    )
    from concourse.dve_spec import Src1  # noqa: F401  (spill target)
    from concourse.dve_uop import DveOpSpec

    have = {op.name: op for op in dmod.OPS}
    if "QNN_SUMWRAP" in have:
        return have["QNN_SUMWRAP"], have["QNN_SINPOLY"]

    # f = t2 - rne(t2); t2 = (x0 + x1)*C0 + C1;  rne via +/- C2 (=MAGIC)
    from concourse.dve_spec import Src1 as S1

    u = Src0 + S1
    t2 = u * C0 + C1
    k = (t2 + C2) - C2
    f = t2 - k

    def ref_sumwrap(in0, in1, s0, s1, imm2):
        u = (in0.astype(np.float32) + in1.astype(np.float32)).astype(np.float32)
        t2 = ((u * np.float32(s0)).astype(np.float32) + np.float32(s1)).astype(
            np.float32
        )
        m = (t2 + np.float32(imm2)).astype(np.float32)
        k = (m - np.float32(imm2)).astype(np.float32)
        return (t2 - k).astype(np.float32)

    # y = f*(c1 + z*(c3 + z*(c5 + z*c7))), z=f^2; c7 rides the C3 slot,
    # spilled to a latched [P,1] in1 read (Src1 as a full stream with a
    # [P,1] AP wedges the DVE -- the stream runs dry after one element)
    z = sq(Src0)
    t = ((z * C3 + C0) * z + C1) * z + C2
    y = _spill_c3_to_src1(t * Src0)

    def ref_sinpoly(in0, in1, s0, s1, imm2):
        x = in0.astype(np.float32)
        z = (x * x).astype(np.float32)
        c7 = np.asarray(in1, np.float32).reshape(-1, 1)
        t = (z * c7).astype(np.float32)
        t = (t + np.float32(s0)).astype(np.float32)
        t = (t * z).astype(np.float32)
        t = (t + np.float32(s1)).astype(np.float32)
        t = (t * z).astype(np.float32)
        t = (t + np.float32(imm2)).astype(np.float32)
        return (t * x).astype(np.float32)

    ops = []
    for name, spec in (
        ("QNN_SUMWRAP", Spec(body=f, reference=ref_sumwrap)),
        ("QNN_SINPOLY", Spec(body=y, reference=ref_sinpoly)),
    ):
        shas = {}
        for ver in ("v3", "v4"):
            s = DveOpSpec(
                name=name, opcode=None, uops=lower(spec, ver=ver),
                rd1_en=_has_src1(spec),
            )
            shas[ver] = s.sha(ver)
        op = DveOp(name, spec, subdim=False, uops_sha=shas)
        row = dmod._CUSTOM_DVE_ROW_BASE + len(dmod.OPS)
        assert row < 0x20
        dmod.OPS.append(op)
        dmod._SUB_OPCODE_FOR_NAME[op.name] = row
        dmod.CUSTOM_DVE_SPECS[op.name] = op.spec
        ops.append(op)
    return ops[0], ops[1]


# ---- plan / scheduling -----------------------------------------------------


def _plan_waits(plan):
    """Assign per-op semaphore waits for every RAW/WAR/WAW hazard."""
    semval = {}
    writer = {}
    readers = {}
    seen = {}
    for op in plan:
        want = {}
        for b in op["reads"]:
            if b in writer:
                s, v = writer[b]
                want[s] = max(want.get(s, 0), v)
        for b in op["writes"]:
            for s, v in readers.get(b, []):
                want[s] = max(want.get(s, 0), v)
            if b in writer:
                s, v = writer[b]
                want[s] = max(want.get(s, 0), v)
        eng_seen = seen.setdefault(op["eng"], {})
        waits = []
        for s, v in want.items():
            if eng_seen.get(s, -1) < v:
                waits.append((s, v))
                eng_seen[s] = v
        op["waits"] = waits
        semval[op["sem"]] = semval.get(op["sem"], 0) + op["inc"]
        point = (op["sem"], semval[op["sem"]])
        for b in op["writes"]:
            writer[b] = point
            readers[b] = []
        for b in op["reads"]:
            readers.setdefault(b, []).append(point)
    return plan


def _build_nc(R: float, phi: float, cfg=None):
    import concourse.bacc as bacc
    from concourse import mybir

    cfg = cfg or CONFIG
    LOAD_COLS = cfg["load_cols"]
    LOAD_RING = cfg["load_ring"]
    SIN_DVE = cfg["sin_dve"]
    MUL_DVE = cfg["mul_dve"]
    STORE_GROUP = cfg["store_group"]
    STORE_RING = cfg["store_ring"]
    NB = len(LOAD_COLS)
    assert sum(LOAD_COLS) == COLS
    off = [sum(LOAD_COLS[:i]) for i in range(NB)]

    SUMWRAP, SINPOLY = _register_ops()

    f16 = mybir.dt.float16
    f32 = mybir.dt.float32
    Sin = mybir.ActivationFunctionType.Sin

    c1, c3, c5, c7 = (c * R for c in SIN_COEF)

    nc = bacc.Bacc(
        "TRN2",
        target_bir_lowering=False,
        debug=False,
        enable_asserts=False,
        num_devices=N_CORES,
    )
    xp = nc.dram_tensor("xp", [2 * B_SHARD], f16, kind="ExternalInput").ap()
    y = nc.dram_tensor("y", [B_SHARD], f16, kind="ExternalOutput").ap()
    xpv = xp.rearrange("(p c) -> p c", p=128)  # [128, 2*COLS], chunk-packed
    yv = y.rearrange("(p c) -> p c", p=128)

    xt = nc.alloc_sbuf_tensor("xt", [128, 2 * COLS], f16)
    ft = nc.alloc_sbuf_tensor("ft", [128, COLS], f32)
    st = nc.alloc_sbuf_tensor("st", [128, COLS], f16)
    yt = nc.alloc_sbuf_tensor("yt", [128, COLS], f16)
    c7t = nc.alloc_sbuf_tensor("c7t", [128, 1], f32)

    def csl(b):
        return slice(off[b], off[b] + LOAD_COLS[b])

    # ---- phase 1: global plan -------------------------------------------
    def op(eng, kind, i, reads, writes, sem, inc=1):
        return dict(eng=eng, kind=kind, i=i, reads=reads, writes=writes,
                    sem=sem, inc=inc)

    plan = []
    for b in range(NB):  # one packed [x0_b | x1_b] load per chunk
        # per-chunk sems: a shared counting sem would be unsafe -- the 16
        # SDMA engines complete their slices of queued DMAs independently,
        # so a total of 16*(b+1) incs does NOT imply chunk b has landed
        plan.append(op(LOAD_RING[b], "ld", b, [], [f"x_{b}"], f"l{b}", 16))
    plan.append(op("v", "memset", 0, [], ["c7t"], "vq"))
    # NOTE: the plan list must be a valid topological execution order --
    # _plan_waits resolves hazards by linear scan.  Engine streams are
    # the per-engine subsequences of this order.  DVE muls are slotted
    # dmul_dist sumwraps behind their block so (a) the ACT sin is done
    # when DVE reaches the mul (no stall) and (b) the mul fills DVE's
    # load-wait gaps, making y blocks -- and their stores -- early.
    DIST = cfg.get("dmul_dist", 2)
    for b in range(NB):
        plan.append(op("v", "sumwrap", b, [f"x_{b}"], [f"f{b}"], "vq"))
        if b in SIN_DVE:  # right after its producer on the same engine
            plan.append(op("v", "sinpoly", b, [f"f{b}", "c7t"], [f"y{b}"], "vq"))
        if b not in SIN_DVE:
            plan.append(op("a", "sin", b, [f"f{b}"], [f"s{b}"], "aq"))
        d = b - DIST
        if d >= 0 and d in MUL_DVE and d not in SIN_DVE:
            plan.append(op("v", "dmul", d, [f"s{d}"], [f"y{d}"], "vq"))
    for d in range(max(0, NB - DIST), NB):
        if d in MUL_DVE and d not in SIN_DVE:
            plan.append(op("v", "dmul", d, [f"s{d}"], [f"y{d}"], "vq"))
    for b in range(NB):
        if b not in SIN_DVE and b not in MUL_DVE:
            plan.append(op("a", "mul", b, [f"s{b}"], [f"y{b}"], "aq"))
    for g, blocks in enumerate(STORE_GROUP):
        plan.append(op(STORE_RING[g], "store", g,
                       [f"y{b}" for b in blocks], [], f"os{g}", 16))

    _plan_waits(plan)

    # ---- phase 2: emit per-engine streams -------------------------------
    with ExitStack() as ctx:
        sems = {}
        for o in plan:
            if o["sem"] not in sems:
                sems[o["sem"]] = ctx.enter_context(nc.semaphore(o["sem"]))
        block = ctx.enter_context(nc.Block())

        def emit(o, eng):
            for s, v in o["waits"]:
                eng.wait_ge(sems[s], v)
            b = o["i"]
            k = o["kind"]
            if k == "ld":
                pc = slice(2 * off[b], 2 * (off[b] + LOAD_COLS[b]))
                inst = eng.dma_start(xt.ap()[:, pc], xpv[:, pc])
            elif k == "store":
                blocks = STORE_GROUP[b]
                c = slice(off[blocks[0]],
                          off[blocks[-1]] + LOAD_COLS[blocks[-1]])
                inst = eng.dma_start(yv[:, c], yt.ap()[:, c])
            elif k == "memset":
                inst = nc.vector.memset(c7t.ap(), c7)
            elif k == "sumwrap":
                c = csl(b)
                lo, cb = 2 * off[b], LOAD_COLS[b]
                inst = nc.vector._custom_dve(
                    SUMWRAP, out=ft.ap()[:, c],
                    in0=xt.ap()[:, lo : lo + cb],
                    in1=xt.ap()[:, lo + cb : lo + 2 * cb],
                    s0=INV_2PI, s1=phi * INV_2PI, imm2=MAGIC,
                )
            elif k == "sinpoly":
                c = csl(b)
                inst = nc.vector._custom_dve(
                    SINPOLY, out=yt.ap()[:, c], in0=ft.ap()[:, c],
                    in1=c7t.ap(), s0=c5, s1=c3, imm2=c1,
                )
            elif k == "dmul":
                c = csl(b)
                inst = nc.vector.tensor_scalar_mul(
                    yt.ap()[:, c], st.ap()[:, c], R
                )
            elif k == "sin":
                c = csl(b)
                inst = nc.scalar.activation(
                    st.ap()[:, c], ft.ap()[:, c], Sin, bias=0.0, scale=TWO_PI
                )
            elif k == "mul":
                c = csl(b)
                inst = nc.scalar.mul(yt.ap()[:, c], st.ap()[:, c], R)
            else:
                raise AssertionError(k)
            inst.then_inc(sems[o["sem"]], o["inc"])

        NG = len(STORE_GROUP)

        def store_waits(eng, ring):
            for g in range(NG):
                if STORE_RING[g] == ring:
                    eng.wait_ge(sems[f"os{g}"], 16)

        @block.sync
        def _(sync):
            for o in plan:
                if o["eng"] == "s":
                    emit(o, sync)
            store_waits(sync, "s")

        @block.vector
        def _(vector):
            for o in plan:
                if o["eng"] == "v":
                    emit(o, vector)

        @block.gpsimd
        def _(gpsimd):
            for o in plan:
                if o["eng"] == "g":
                    emit(o, gpsimd)
            store_waits(gpsimd, "g")

        @block.scalar
        def _(scalar):
            for o in plan:
                if o["eng"] == "a":
                    emit(o, scalar)
            store_waits(scalar, "a")

    nc.compile()
    return nc


def kernel(inputs: np.ndarray, weights: np.ndarray, _trace: bool = False,
           _cfg=None) -> np.ndarray:
    global LAST_RESULT
    from concourse.bass_utils import run_bass_kernel_spmd

    inputs = np.asarray(inputs, dtype=np.float32)
    assert inputs.shape == (B_FULL, 2), inputs.shape

    R, phi = _host_constants(weights)
    nc = _build_nc(R, phi, _cfg)

    cfg = _cfg or CONFIG
    load_cols = cfg["load_cols"]
    off = [sum(load_cols[:i]) for i in range(len(load_cols))]
    xh = inputs.astype(np.float16)
    in_maps = []
    for c in range(N_CORES):
        sh = xh[c * B_SHARD : (c + 1) * B_SHARD]
        x0m = sh[:, 0].reshape(128, COLS)
        x1m = sh[:, 1].reshape(128, COLS)
        packed = np.empty((128, 2 * COLS), np.float16)
        for b, cb in enumerate(load_cols):
            o = off[b]
            packed[:, 2 * o : 2 * o + cb] = x0m[:, o : o + cb]
            packed[:, 2 * o + cb : 2 * o + 2 * cb] = x1m[:, o : o + cb]
        in_maps.append({"xp": packed.ravel()})
    res = run_bass_kernel_spmd(
        nc, in_maps, core_ids=list(range(N_CORES)), trace=_trace
    )
    LAST_RESULT = res
    out = np.concatenate([r["y"] for r in res.results], axis=0)
    return out.astype(np.float32).reshape(B_FULL, 1)


# revision 24
# speedup vs baseline: 1.7458x; 1.0008x over previous
"""Trainium2 Bass kernel for nn_EstimatorQNN.

Math reduction: the reference applies a batch-independent 2x2 unitary U
(built from the 4 weights) to |psi> = [cos(th/2), sin(th/2)] with
th = x0 + x1, then returns |amp0|^2 - |amp1|^2.  By unitarity this
collapses to

    out = R*sin(th + phi)

with R = hypot(A, D), phi = atan2(A, D), A = 2|U00|^2 - 1,
D = 2*Re(U00*conj(U01)) -- 2 scalars computed on host from the weights;
the device does the memory-bound elementwise part.

HBM traffic is halved vs f32 by staging the inputs as de-interleaved
fp16 streams and storing fp16 (host widens to f32; tolerance is 2e-2,
fp16 costs ~5e-4).  Per core: 4 MB loads + 2 MB stores = 6 MB.

Compute is two fused custom-DVE ops (registered via the documented
dve_ops.OPS extension point; shas self-pinned at import):

  QNN_SUMWRAP:  f = t2 - rne_round(t2),  t2 = (x0+x1)*(1/2pi) + phi/2pi
                (round via the fp32 +/-1.5*2^23 magic trick).  f is the
                angle in *turns*, exactly in [-0.5, 0.5] for all inputs
                -- no range-reduction tail cases.  1 elem/lane/cycle.
  QNN_SINPOLY:  y = R*sin(2pi*f) as an odd minimax degree-7 polynomial
                (R folded into the coefficients, max err 2.5e-4); c7
                rides the C3 slot latched from a [P,1] in1.

Per column-block: DVE runs SUMWRAP; ACT evaluates sin via its table
(free input scale=2pi) and DVE applies the R-mul as a stock fp16
tensor_scalar (~3x faster than an ACT Copy), slotted one sumwrap
behind its block so it fills DVE's load-wait gaps and the y blocks --
and their stores -- land early enough to overlap the load phase.  The
two tail blocks use DVE's SINPOLY instead (shortest chunk->store
chain, no ACT round trip).  Raw-Bass hand-scheduled: loads alternate
between the two HWDGE rings with small first chunks (early first
compute) and small last chunks (short tail); paired stores on the
gpsimd/sync rings; explicit semaphore waits for every hazard.  The
steady state runs at the per-core HBM roofline (~350 GB/s): chunk
cadence ~1.45 us per 512 KB.  Pure data parallel over 8 NeuronCores.
"""

import math
from contextlib import ExitStack

import numpy as np

B_FULL = 8388608
N_CORES = 8
B_SHARD = B_FULL // N_CORES  # 1048576
COLS = B_SHARD // 128        # 8192 cols per partition

# Block layout: load_cols lists the load chunks; compute/store blocks
# subdivide them at block_max (1024) cols -- at these sizes they are
# 1:1, which measured fastest (bigger chunks coarsen the pipeline and
# lose more than their fewer-DMAs save; 512-col edge chunks shorten
# ramp and tail).  SIN_DVE blocks evaluate sin via QNN_SINPOLY on DVE
# (no separate mul); others use ACT Sin + a DVE fp16 tensor_scalar mul
# (MUL_DVE).  Indices in sin_dve/mul_dve/store_group refer to blocks.
# Inputs are host-packed per chunk: [x0_c rows | x1_c rows] so each
# chunk is ONE [128, 2*cols] DMA with 2*cols*2B contiguous per-partition
# runs (4KB descriptors at 1024 cols -- ~25% more engine bandwidth than
# the 2KB ones two separate x0/x1 loads would give).
CONFIG = dict(
    load_cols=[512, 512, 1024, 1024, 1024, 1024, 1024, 1024, 512, 512],
    load_ring="sasasasasa",      # per-chunk dispatch ring
    sin_dve={8, 9},              # tail blocks: shortest path to the store
    mul_dve={0, 1, 2, 3, 4, 5, 6, 7},
    store_group=[(0, 1), (2, 3), (4, 5), (6, 7), (8,), (9,)],
    store_ring=["g", "s", "g", "s", "g", "s"],
    dmul_dist=1,
)

MAGIC = 12582912.0           # 1.5 * 2**23: fp32 round-to-nearest-even
TWO_PI = 6.283185307179586
INV_2PI = 1.0 / TWO_PI
# minimax odd deg-7 fit of sin(2*pi*f) on f in [-0.5, 0.5], max err 2.5e-4
SIN_COEF = (6.27864315, -41.09402942, 77.93314409, -56.09367861)

LAST_RESULT = None


def _host_constants(weights: np.ndarray):
    w = np.asarray(weights, dtype=np.float64)

    def rx(t):
        c, s = np.cos(t / 2), np.sin(t / 2)
        return np.array([[c, -1j * s], [-1j * s, c]], dtype=np.complex128)

    def rz(t):
        return np.array(
            [[np.exp(-1j * t / 2), 0], [0, np.exp(1j * t / 2)]], dtype=np.complex128
        )

    U = np.eye(2, dtype=np.complex128)
    for i in range(len(w) // 2):
        U = rz(w[2 * i + 1]) @ rx(w[2 * i]) @ U
    A = 2.0 * abs(U[0, 0]) ** 2 - 1.0
    D = 2.0 * (U[0, 0] * np.conj(U[0, 1])).real
    R = math.hypot(A, D)
    phi = math.atan2(A, D)
    return float(R), float(phi)


# ---- custom DVE ops --------------------------------------------------------


def _register_ops():
    """Define + register the two fused DVE ops (idempotent).  uops_sha is
    self-pinned from lower() so concourse-version drift can't stale-pin;
    numerics are verified against the reference by the test harness."""
    import concourse.dve_ops as dmod
    from concourse.dve_ops import DveOp
    from concourse.dve_spec import (
        C0, C1, C2, C3, Spec, Src0, _has_src1, _spill_c3_to_src1, lower, sq,
    )
    from concourse.dve_spec import Src1  # noqa: F401  (spill target)
    from concourse.dve_uop import DveOpSpec

    have = {op.name: op for op in dmod.OPS}
    if "QNN_SUMWRAP" in have:
        return have["QNN_SUMWRAP"], have["QNN_SINPOLY"]

    # f = t2 - rne(t2); t2 = (x0 + x1)*C0 + C1;  rne via +/- C2 (=MAGIC)
    from concourse.dve_spec import Src1 as S1

    u = Src0 + S1
    t2 = u * C0 + C1
    k = (t2 + C2) - C2
    f = t2 - k

    def ref_sumwrap(in0, in1, s0, s1, imm2):
        u = (in0.astype(np.float32) + in1.astype(np.float32)).astype(np.float32)
        t2 = ((u * np.float32(s0)).astype(np.float32) + np.float32(s1)).astype(
            np.float32
        )
        m = (t2 + np.float32(imm2)).astype(np.float32)
        k = (m - np.float32(imm2)).astype(np.float32)
        return (t2 - k).astype(np.float32)

    # y = f*(c1 + z*(c3 + z*(c5 + z*c7))), z=f^2; c7 rides the C3 slot,
    # spilled to a latched [P,1] in1 read (Src1 as a full stream with a
    # [P,1] AP wedges the DVE -- the stream runs dry after one element)
    z = sq(Src0)
    t = ((z * C3 + C0) * z + C1) * z + C2
    y = _spill_c3_to_src1(t * Src0)

    def ref_sinpoly(in0, in1, s0, s1, imm2):
        x = in0.astype(np.float32)
        z = (x * x).astype(np.float32)
        c7 = np.asarray(in1, np.float32).reshape(-1, 1)
        t = (z * c7).astype(np.float32)
        t = (t + np.float32(s0)).astype(np.float32)
        t = (t * z).astype(np.float32)
        t = (t + np.float32(s1)).astype(np.float32)
        t = (t * z).astype(np.float32)
        t = (t + np.float32(imm2)).astype(np.float32)
        return (t * x).astype(np.float32)

    ops = []
    for name, spec in (
        ("QNN_SUMWRAP", Spec(body=f, reference=ref_sumwrap)),
        ("QNN_SINPOLY", Spec(body=y, reference=ref_sinpoly)),
    ):
        shas = {}
        for ver in ("v3", "v4"):
            s = DveOpSpec(
                name=name, opcode=None, uops=lower(spec, ver=ver),
                rd1_en=_has_src1(spec),
            )
            shas[ver] = s.sha(ver)
        op = DveOp(name, spec, subdim=False, uops_sha=shas)
        row = dmod._CUSTOM_DVE_ROW_BASE + len(dmod.OPS)
        assert row < 0x20
        dmod.OPS.append(op)
        dmod._SUB_OPCODE_FOR_NAME[op.name] = row
        dmod.CUSTOM_DVE_SPECS[op.name] = op.spec
        ops.append(op)
    return ops[0], ops[1]


# ---- plan / scheduling -----------------------------------------------------


def _plan_waits(plan):
    """Assign per-op semaphore waits for every RAW/WAR/WAW hazard."""
    semval = {}
    writer = {}
    readers = {}
    seen = {}
    for op in plan:
        want = {}
        for b in op["reads"]:
            if b in writer:
                s, v = writer[b]
                want[s] = max(want.get(s, 0), v)
        for b in op["writes"]:
            for s, v in readers.get(b, []):
                want[s] = max(want.get(s, 0), v)
            if b in writer:
                s, v = writer[b]
                want[s] = max(want.get(s, 0), v)
        eng_seen = seen.setdefault(op["eng"], {})
        waits = []
        for s, v in want.items():
            if eng_seen.get(s, -1) < v:
                waits.append((s, v))
                eng_seen[s] = v
        op["waits"] = waits
        semval[op["sem"]] = semval.get(op["sem"], 0) + op["inc"]
        point = (op["sem"], semval[op["sem"]])
        for b in op["writes"]:
            writer[b] = point
            readers[b] = []
        for b in op["reads"]:
            readers.setdefault(b, []).append(point)
    return plan


def _build_nc(R: float, phi: float, cfg=None):
    import concourse.bacc as bacc
    from concourse import mybir

    cfg = cfg or CONFIG
    LOAD_COLS = cfg["load_cols"]
    LOAD_RING = cfg["load_ring"]
    SIN_DVE = cfg["sin_dve"]
    MUL_DVE = cfg["mul_dve"]
    STORE_GROUP = cfg["store_group"]
    STORE_RING = cfg["store_ring"]
    NC_ = len(LOAD_COLS)
    assert sum(LOAD_COLS) == COLS
    coff = [sum(LOAD_COLS[:i]) for i in range(NC_)]
    # Compute blocks subdivide load chunks (chunks can be larger than a
    # block: HWDGE descriptor GENERATION is ~fixed per DMA -- 128 descs
    # at ~23 ns -- so bigger chunks double bytes-per-gen while DVE still
    # consumes 1024-col pieces).  cfg indices refer to BLOCKS.
    BMAX = cfg.get("block_max", 1024)
    BLK = []          # (cols, block_off, chunk_idx)
    for ci, cc in enumerate(LOAD_COLS):
        o = coff[ci]
        while cc > 0:
            take = min(cc, BMAX)
            BLK.append((take, o, ci))
            o += take
            cc -= take
    NB = len(BLK)
    off = [b[1] for b in BLK]
    BCOLS = [b[0] for b in BLK]
    B2C = [b[2] for b in BLK]

    SUMWRAP, SINPOLY = _register_ops()

    f16 = mybir.dt.float16
    f32 = mybir.dt.float32
    Sin = mybir.ActivationFunctionType.Sin

    c1, c3, c5, c7 = (c * R for c in SIN_COEF)

    nc = bacc.Bacc(
        "TRN2",
        target_bir_lowering=False,
        debug=False,
        enable_asserts=False,
        num_devices=N_CORES,
    )
    xp = nc.dram_tensor("xp", [2 * B_SHARD], f16, kind="ExternalInput").ap()
    y = nc.dram_tensor("y", [B_SHARD], f16, kind="ExternalOutput").ap()
    xpv = xp.rearrange("(p c) -> p c", p=128)  # [128, 2*COLS], chunk-packed
    yv = y.rearrange("(p c) -> p c", p=128)

    xt = nc.alloc_sbuf_tensor("xt", [128, 2 * COLS], f16)
    ft = nc.alloc_sbuf_tensor("ft", [128, COLS], f32)
    st = nc.alloc_sbuf_tensor("st", [128, COLS], f16)
    yt = nc.alloc_sbuf_tensor("yt", [128, COLS], f16)
    c7t = nc.alloc_sbuf_tensor("c7t", [128, 1], f32)

    def csl(b):
        return slice(off[b], off[b] + BCOLS[b])

    # ---- phase 1: global plan -------------------------------------------
    def op(eng, kind, i, reads, writes, sem, inc=1):
        return dict(eng=eng, kind=kind, i=i, reads=reads, writes=writes,
                    sem=sem, inc=inc)

    plan = []
    for c in range(NC_):  # one packed [x0_c | x1_c] load per chunk
        # per-chunk sems: a shared counting sem would be unsafe -- the 16
        # SDMA engines complete their slices of queued DMAs independently,
        # so a total of 16*(c+1) incs does NOT imply chunk c has landed
        plan.append(op(LOAD_RING[c], "ld", c, [], [f"x_{c}"], f"l{c}", 16))
    plan.append(op("v", "memset", 0, [], ["c7t"], "vq"))
    # NOTE: the plan list must be a valid topological execution order --
    # _plan_waits resolves hazards by linear scan.  Engine streams are
    # the per-engine subsequences of this order.  DVE muls are slotted
    # dmul_dist sumwraps behind their block so (a) the ACT sin is done
    # when DVE reaches the mul (no stall) and (b) the mul fills DVE's
    # load-wait gaps, making y blocks -- and their stores -- early.
    DIST = cfg.get("dmul_dist", 2)
    for b in range(NB):
        plan.append(op("v", "sumwrap", b, [f"x_{B2C[b]}"], [f"f{b}"], "vq"))
        if b in SIN_DVE:  # right after its producer on the same engine
            plan.append(op("v", "sinpoly", b, [f"f{b}", "c7t"], [f"y{b}"], "vq"))
        if b not in SIN_DVE:
            plan.append(op("a", "sin", b, [f"f{b}"], [f"s{b}"], "aq"))
        d = b - DIST
        if d >= 0 and d in MUL_DVE and d not in SIN_DVE:
            plan.append(op("v", "dmul", d, [f"s{d}"], [f"y{d}"], "vq"))
    for d in range(max(0, NB - DIST), NB):
        if d in MUL_DVE and d not in SIN_DVE:
            plan.append(op("v", "dmul", d, [f"s{d}"], [f"y{d}"], "vq"))
    for b in range(NB):
        if b not in SIN_DVE and b not in MUL_DVE:
            plan.append(op("a", "mul", b, [f"s{b}"], [f"y{b}"], "aq"))
    for g, blocks in enumerate(STORE_GROUP):
        plan.append(op(STORE_RING[g], "store", g,
                       [f"y{b}" for b in blocks], [], f"os{g}", 16))

    _plan_waits(plan)

    # ---- phase 2: emit per-engine streams -------------------------------
    with ExitStack() as ctx:
        sems = {}
        for o in plan:
            if o["sem"] not in sems:
                sems[o["sem"]] = ctx.enter_context(nc.semaphore(o["sem"]))
        block = ctx.enter_context(nc.Block())

        def emit(o, eng):
            for s, v in o["waits"]:
                eng.wait_ge(sems[s], v)
            b = o["i"]
            k = o["kind"]
            if k == "ld":
                pc = slice(2 * coff[b], 2 * (coff[b] + LOAD_COLS[b]))
                inst = eng.dma_start(xt.ap()[:, pc], xpv[:, pc])
            elif k == "store":
                blocks = STORE_GROUP[b]
                c = slice(off[blocks[0]],
                          off[blocks[-1]] + BCOLS[blocks[-1]])
                inst = eng.dma_start(yv[:, c], yt.ap()[:, c])
            elif k == "memset":
                inst = nc.vector.memset(c7t.ap(), c7)
            elif k == "sumwrap":
                c = csl(b)
                ci = B2C[b]
                d = off[b] - coff[ci]       # block offset inside its chunk
                lo, cb = 2 * coff[ci] + d, BCOLS[b]
                cw = LOAD_COLS[ci]          # x1 half starts cw cols in
                inst = nc.vector._custom_dve(
                    SUMWRAP, out=ft.ap()[:, c],
                    in0=xt.ap()[:, lo : lo + cb],
                    in1=xt.ap()[:, lo + cw : lo + cw + cb],
                    s0=INV_2PI, s1=phi * INV_2PI, imm2=MAGIC,
                )
            elif k == "sinpoly":
                c = csl(b)
                inst = nc.vector._custom_dve(
                    SINPOLY, out=yt.ap()[:, c], in0=ft.ap()[:, c],
                    in1=c7t.ap(), s0=c5, s1=c3, imm2=c1,
                )
            elif k == "dmul":
                c = csl(b)
                inst = nc.vector.tensor_scalar_mul(
                    yt.ap()[:, c], st.ap()[:, c], R
                )
            elif k == "sin":
                c = csl(b)
                inst = nc.scalar.activation(
                    st.ap()[:, c], ft.ap()[:, c], Sin, bias=0.0, scale=TWO_PI
                )
            elif k == "mul":
                c = csl(b)
                inst = nc.scalar.mul(yt.ap()[:, c], st.ap()[:, c], R)
            else:
                raise AssertionError(k)
            inst.then_inc(sems[o["sem"]], o["inc"])

        NG = len(STORE_GROUP)

        def store_waits(eng, ring):
            for g in range(NG):
                if STORE_RING[g] == ring:
                    eng.wait_ge(sems[f"os{g}"], 16)

        @block.sync
        def _(sync):
            for o in plan:
                if o["eng"] == "s":
                    emit(o, sync)
            store_waits(sync, "s")

        @block.vector
        def _(vector):
            for o in plan:
                if o["eng"] == "v":
                    emit(o, vector)

        @block.gpsimd
        def _(gpsimd):
            for o in plan:
                if o["eng"] == "g":
                    emit(o, gpsimd)
            store_waits(gpsimd, "g")

        @block.scalar
        def _(scalar):
            for o in plan:
                if o["eng"] == "a":
                    emit(o, scalar)
            store_waits(scalar, "a")

    nc.compile()
    return nc


def kernel(inputs: np.ndarray, weights: np.ndarray, _trace: bool = False,
           _cfg=None) -> np.ndarray:
    global LAST_RESULT
    from concourse.bass_utils import run_bass_kernel_spmd

    inputs = np.asarray(inputs, dtype=np.float32)
    assert inputs.shape == (B_FULL, 2), inputs.shape

    R, phi = _host_constants(weights)
    nc = _build_nc(R, phi, _cfg)

    cfg = _cfg or CONFIG
    load_cols = cfg["load_cols"]
    off = [sum(load_cols[:i]) for i in range(len(load_cols))]
    xh = inputs.astype(np.float16)
    in_maps = []
    for c in range(N_CORES):
        sh = xh[c * B_SHARD : (c + 1) * B_SHARD]
        x0m = sh[:, 0].reshape(128, COLS)
        x1m = sh[:, 1].reshape(128, COLS)
        packed = np.empty((128, 2 * COLS), np.float16)
        for b, cb in enumerate(load_cols):
            o = off[b]
            packed[:, 2 * o : 2 * o + cb] = x0m[:, o : o + cb]
            packed[:, 2 * o + cb : 2 * o + 2 * cb] = x1m[:, o : o + cb]
        in_maps.append({"xp": packed.ravel()})
    res = run_bass_kernel_spmd(
        nc, in_maps, core_ids=list(range(N_CORES)), trace=_trace
    )
    LAST_RESULT = res
    out = np.concatenate([r["y"] for r in res.results], axis=0)
    return out.astype(np.float32).reshape(B_FULL, 1)
